# revision 18
# baseline (speedup 1.0000x reference)
"""Trainium2 Bass kernel for nn_AttentionWithEpinions (GNN edge attention with
segment softmax over destination nodes), 8 NeuronCores.

Strategy (graph partitioning by destination node, per the sharding hint):
- Host sorts edges by destination and bin-packs whole destination segments
  into 1024 partition-rows (8 devices x 128 rows x F slots). Every segment is
  fully contained in one partition-row of one device, so the segment softmax
  is entirely local: no collectives. The per-edge operand streams (src
  features and the destination feature of each edge) are laid out by the host
  in a feature-on-partitions [d, e] fp16 layout.
- Per device, per 1024-slot superblock:
    score^T = W_src^T @ src^T  (+)  W_dst^T @ dstf^T       (PSUM accumulate)
    a1 = Lrelu(score + b_src + b_dst)                      (evict to fp16)
    h  = W1^T @ a1                                         (PSUM)
    a2 = Lrelu(h + b1)                                     (fp16)
    logits-accumulation: col-tiled M=32 matmuls with a one-hot-padded w2;
      17 superblocks' logits land in distinct rows of one PSUM bank
      (other rows accumulate exact zeros), evicted once per block.
- Segment softmax via segmented scans on the [128, F] slot grid:
  forward scan accumulates per-segment sums, a reversed scan broadcasts each
  segment's total back to its slots, then attn = ex / total. exp() uses a
  constant shift instead of the per-segment max (cancels exactly in the
  softmax ratio; logits are O(+-5) so fp32 exp is safe).
"""

import os
import numpy as np

import concourse.bass as bass
import concourse.mybir as mybir
import concourse.tile as tile
from concourse import bacc
from concourse.bass_utils import run_bass_kernel_spmd

# ---------------- compile-time configuration ----------------
D = 128
CORES = 8
F = 1632                  # slots per partition row
EPAD = 128 * F            # 208896 slots per device
SB = 1024                 # superblock (slots) flowing through PSUM together
NSB = EPAD // SB          # 204
TILE = 512                # matmul moving free dim
LGB = 17                  # superblocks whose logits accumulate into one PSUM bank
NLGB = NSB // LGB         # 12 logit blocks
SHIFT = 16.0              # exp() stability shift (cancels in the softmax)
N_NODES = 50000
N_EDGES = 1600000

f32 = mybir.dt.float32
f16 = mybir.dt.float16

Lrelu = mybir.ActivationFunctionType.Lrelu
Exp = mybir.ActivationFunctionType.Exp
Copy = mybir.ActivationFunctionType.Copy
ADD = mybir.AluOpType.add
MULT = mybir.AluOpType.mult
MAX = mybir.AluOpType.max


def _evict_on_dve(evict_idx):
    """Balance knob: which of the 2*NSB big PSUM evictions run on DVE
    (biased copy, LReLU applied by a second op on DVE or GPSIMD) vs ACT
    (1 fused op). ACT is faster per eviction, so it gets ~5/9."""
    return evict_idx % 13 in (1, 4, 7, 10)


def build_nc():
    nc = bacc.Bacc("TRN2", target_bir_lowering=False, debug=False)

    srcT_d = nc.dram_tensor("srcT", [128, EPAD], f16, kind="ExternalInput")
    dstT_d = nc.dram_tensor("dstT", [128, EPAD], f16, kind="ExternalInput")
    flags_d = nc.dram_tensor("flags", [128, F], f32, kind="ExternalInput")
    endm_d = nc.dram_tensor("endm", [128, F], f32, kind="ExternalInput")
    fbwd_d = nc.dram_tensor("fbwd", [128, F], f32, kind="ExternalInput")
    wsrc_d = nc.dram_tensor("wsrc", [D, D], f16, kind="ExternalInput")
    wdst_d = nc.dram_tensor("wdst", [D, D], f16, kind="ExternalInput")
    w1_d = nc.dram_tensor("w1", [D, D], f16, kind="ExternalInput")
    w2pad_d = nc.dram_tensor("w2pad", [D, LGB * 32], f16, kind="ExternalInput")
    bsum_d = nc.dram_tensor("bsum", [D, 1], f32, kind="ExternalInput")
    b1_d = nc.dram_tensor("b1", [D, 1], f32, kind="ExternalInput")
    bexp_d = nc.dram_tensor("bexp", [D, 1], f32, kind="ExternalInput")

    out_d = nc.dram_tensor("out", [128, F], f32, kind="ExternalOutput")
    lg_d = nc.dram_tensor("lg_scratch", [EPAD], f32)  # internal DRAM staging

    TPS = SB // TILE  # tiles per superblock (2)

    with tile.TileContext(nc) as tc:
        with tc.tile_pool(name="const", bufs=1) as cst:
            wsrc_s = cst.tile([D, D], f16)
            wdst_s = cst.tile([D, D], f16)
            w1_s = cst.tile([D, D], f16)
            w2pad_s = cst.tile([D, LGB * 32], f16)
            bsum_s = cst.tile([D, 1], f32)
            b1_s = cst.tile([D, 1], f32)
            bexp_s = cst.tile([D, 1], f32)
            for s, d in [(wsrc_s, wsrc_d), (wdst_s, wdst_d), (w1_s, w1_d),
                         (w2pad_s, w2pad_d), (bsum_s, bsum_d), (b1_s, b1_d),
                         (bexp_s, bexp_d)]:
                nc.sync.dma_start(s[:], d[:])

            # ---------------- phase 1: per-edge MLP -> logits ----------------
            with tc.tile_pool(name="stream", bufs=3) as stp, \
                 tc.tile_pool(name="act", bufs=4) as actp, \
                 tc.tile_pool(name="lgst", bufs=2) as lgstp, \
                 tc.tile_pool(name="ps", bufs=3, space="PSUM") as psp, \
                 tc.tile_pool(name="pslg", bufs=2, space="PSUM") as pslgp:
                lgp = None
                st4 = dt4 = None
                for sb in range(NSB):
                    if sb % 4 == 0:
                        o4 = sb * SB
                        st4 = stp.tile([128, 4 * SB], f16, tag="st4")
                        nc.sync.dma_start(st4[:], srcT_d[:, o4 : o4 + 4 * SB])
                        dt4 = stp.tile([128, 4 * SB], f16, tag="dt4")
                        nc.sync.dma_start(dt4[:], dstT_d[:, o4 : o4 + 4 * SB])
                    st = st4[:, (sb % 4) * SB : (sb % 4 + 1) * SB]
                    dt = dt4[:, (sb % 4) * SB : (sb % 4 + 1) * SB]

                    score = psp.tile([128, SB], f32, tag="ps")
                    for t in range(TPS):
                        nc.tensor.matmul(
                            score[:, t * TILE : (t + 1) * TILE],
                            wsrc_s[:], st[:, t * TILE : (t + 1) * TILE],
                            start=True, stop=False)
                    for t in range(TPS):
                        nc.tensor.matmul(
                            score[:, t * TILE : (t + 1) * TILE],
                            wdst_s[:], dt[:, t * TILE : (t + 1) * TILE],
                            start=False, stop=True)

                    a1 = actp.tile([128, SB], f16, tag="a1")
                    ei = 2 * sb
                    if _evict_on_dve(ei):
                        nc.vector.tensor_scalar(a1[:], score[:], bsum_s[:], None, ADD)
                        nc.vector.scalar_tensor_tensor(a1[:], a1[:], 0.01, a1[:], MULT, MAX)
                    else:
                        nc.scalar.activation(a1[:], score[:], Lrelu,
                                             bias=bsum_s[:], scale=1.0, alpha=0.01)

                    h = psp.tile([128, SB], f32, tag="ps")
                    for t in range(TPS):
                        nc.tensor.matmul(
                            h[:, t * TILE : (t + 1) * TILE],
                            w1_s[:], a1[:, t * TILE : (t + 1) * TILE],
                            start=True, stop=True)

                    a2 = actp.tile([128, SB], f16, tag="a2")
                    ei = 2 * sb + 1
                    if _evict_on_dve(ei):
                        nc.vector.tensor_scalar(a2[:], h[:], b1_s[:], None, ADD)
                        nc.vector.scalar_tensor_tensor(a2[:], a2[:], 0.01, a2[:], MULT, MAX)
                    else:
                        nc.scalar.activation(a2[:], h[:], Lrelu,
                                             bias=b1_s[:], scale=1.0, alpha=0.01)

                    # logits: col-tiled accumulating matmuls, M=32 with a
                    # one-hot-padded w2 (column k = sb % LGB). Row 32t+k of
                    # the shared bank receives this superblock's tile-t
                    # logits; all other rows accumulate exact zeros.
                    k = sb % LGB
                    if k == 0:
                        lgp = pslgp.tile([128, TILE], f32, tag="lg")
                    for t in range(TPS):
                        nc.tensor.matmul(
                            lgp[32 * t : 32 * t + 32, :],
                            w2pad_s[:, 32 * k : 32 * (k + 1)],
                            a2[:, t * TILE : (t + 1) * TILE],
                            start=(k == 0), stop=(k == LGB - 1),
                            tile_position=(0, 32 * t))
                    if k == LGB - 1:
                        blk = sb // LGB
                        lgs = lgstp.tile([64, TILE], f32, tag="lgs")
                        nc.scalar.activation(lgs[:], lgp[0:64, :], Copy)
                        # superblock j of this block, tile t lives at
                        # lgs[32*t + j, :] -> lg_d[(blk*LGB+j)*SB + t*TILE :+TILE]
                        lgv = lg_d[:].rearrange("(j t f) -> j t f", t=TPS, f=TILE)
                        for t in range(TPS):
                            nc.sync.dma_start(
                                lgv[blk * LGB : (blk + 1) * LGB, t, :],
                                lgs[32 * t : 32 * t + LGB, :])

            # ---------------- phase 2: segment softmax ----------------
            with tc.tile_pool(name="soft", bufs=1) as sfp:
                lgsc = sfp.tile([128, F], f32)
                nc.sync.dma_start(lgsc[:], lg_d[:].rearrange("(p f) -> p f", p=128))
                flags_s = sfp.tile([128, F], f32)
                endm_s = sfp.tile([128, F], f32)
                fbwd_s = sfp.tile([128, F], f32)
                nc.sync.dma_start(flags_s[:], flags_d[:])
                nc.sync.dma_start(endm_s[:], endm_d[:])
                nc.sync.dma_start(fbwd_s[:], fbwd_d[:])

                ex = sfp.tile([128, F], f32)
                nc.scalar.activation(ex[:], lgsc[:], Exp, bias=bexp_s[:], scale=1.0)

                S = sfp.tile([128, F], f32)
                nc.vector.tensor_tensor_scan(S[:], flags_s[:], ex[:], 0.0, MULT, ADD)
                dend = sfp.tile([128, F], f32)
                nc.vector.tensor_tensor(dend[:], S[:], endm_s[:], MULT)
                Trev = sfp.tile([128, F], f32)
                nc.vector.tensor_tensor_scan(Trev[:], fbwd_s[:], dend[:, ::-1], 0.0, MULT, ADD)
                R = sfp.tile([128, F], f32)
                nc.vector.reciprocal(R[:], Trev[:])
                attn = sfp.tile([128, F], f32)
                nc.vector.tensor_tensor(attn[:], ex[:], R[:, ::-1], MULT)
                nc.sync.dma_start(out_d[:], attn[:])

    nc.finalize()
    return nc


# ---------------- host-side packing ----------------

def _pack(edge_dst):
    order = np.argsort(edge_dst, kind="stable")
    sdst = edge_dst[order].astype(np.int64)
    counts = np.bincount(edge_dst, minlength=N_NODES).astype(np.int64)

    row_of_node = np.empty(N_NODES, np.int64)
    col_of_node = np.empty(N_NODES, np.int64)
    row, col = 0, 0
    for n in range(N_NODES):
        c = counts[n]
        if col + c > F:
            row += 1
            col = 0
        row_of_node[n] = row
        col_of_node[n] = col
        col += c
    assert row < 128 * CORES, f"packing overflow: {row}"

    starts = np.cumsum(counts) - counts
    within = np.arange(N_EDGES, dtype=np.int64) - starts[sdst]
    slot_global = row_of_node[sdst] * F + col_of_node[sdst] + within
    dev_of_edge = (row_of_node[sdst] // 128).astype(np.int64)
    slot_in_dev = slot_global - dev_of_edge * EPAD
    return dict(order=order, sdst=sdst, dev_of_edge=dev_of_edge,
                slot_in_dev=slot_in_dev)


def _device_inputs(P, src, dstf, edge_dst, d):
    mask = P["dev_of_edge"] == d
    slots = P["slot_in_dev"][mask]
    eids = P["order"][mask]

    srcT = np.zeros((EPAD, D), np.float16)
    srcT[slots] = src[eids].astype(np.float16)
    srcT = np.ascontiguousarray(srcT.T)

    dstT = np.zeros((EPAD, D), np.float16)
    dstT[slots] = dstf[edge_dst[eids]].astype(np.float16)
    dstT = np.ascontiguousarray(dstT.T)

    used = np.zeros(EPAD, bool)
    used[slots] = True
    fl = np.ones(EPAD, np.float32)
    sd = P["sdst"][mask]
    seg_start_slots = slots[np.concatenate([[True], sd[1:] != sd[:-1]])]
    fl[seg_start_slots] = 0.0
    prev_used = np.concatenate([[False], used[:-1]])
    run_start = (~used) & (prev_used | (np.arange(EPAD) % F == 0))
    fl[run_start] = 0.0
    fl[np.arange(0, EPAD, F)] = 0.0
    flags = fl.reshape(128, F)

    nxt_reset = np.concatenate([flags[:, 1:], np.zeros((128, 1), np.float32)], axis=1)
    endm = np.where(nxt_reset == 0.0, 1.0, 0.0).astype(np.float32)
    fbwd = np.ascontiguousarray((1.0 - endm)[:, ::-1])

    return dict(srcT=srcT, dstT=dstT, flags=flags, endm=endm, fbwd=fbwd), slots, eids


_CACHE = {}


def run(inputs, trace=False):
    src = np.asarray(inputs["src_feat"], np.float32)
    dstf = np.asarray(inputs["dst_feat"], np.float32)
    edge_dst = np.asarray(inputs["edge_dst"]).astype(np.int64)
    assert src.shape == (N_EDGES, D) and dstf.shape == (N_NODES, D)

    P = _pack(edge_dst)

    wsrc = np.asarray(inputs["W_src"], np.float32).astype(np.float16)
    wdst = np.asarray(inputs["W_dst"], np.float32).astype(np.float16)
    w1 = np.asarray(inputs["W1"], np.float32).astype(np.float16)
    w2v = np.asarray(inputs["W2"], np.float32).astype(np.float16).reshape(D)
    w2pad = np.zeros((D, LGB * 32), np.float16)
    for k in range(LGB):
        w2pad[:, 32 * k + k] = w2v
    bsum = (np.asarray(inputs["b_src"], np.float32)
            + np.asarray(inputs["b_dst"], np.float32)).reshape(D, 1)
    b1 = np.asarray(inputs["b1"], np.float32).reshape(D, 1)
    bexp = np.full((D, 1), float(np.asarray(inputs["b2"]).reshape(-1)[0]) - SHIFT,
                   np.float32)

    in_maps = []
    recov = []
    for d in range(CORES):
        dv, slots, eids = _device_inputs(P, src, dstf, edge_dst, d)
        dv.update(wsrc=wsrc, wdst=wdst, w1=w1, w2pad=w2pad, bsum=bsum, b1=b1,
                  bexp=bexp)
        in_maps.append(dv)
        recov.append((slots, eids))

    if "nc" not in _CACHE:
        _CACHE["nc"] = build_nc()
    nc = _CACHE["nc"]

    res = run_bass_kernel_spmd(nc, in_maps, list(range(CORES)), trace=trace)

    out = np.empty(N_EDGES, np.float32)
    for d in range(CORES):
        slots, eids = recov[d]
        vals = np.asarray(res.results[d]["out"], np.float32).reshape(-1)
        out[eids] = vals[slots]
    _CACHE["exec_time_ns"] = res.exec_time_ns
    _CACHE["trace_path"] = (res.instructions_and_trace or (None, None))[1]
    return out[:, None]


def kernel(**inputs):
    return run(inputs, trace=bool(os.environ.get("BASS_TRACE")))


# revision 19
# speedup vs baseline: 1.1437x; 1.1437x over previous
"""Trainium2 Bass kernel for nn_AttentionWithEpinions (GNN edge attention with
segment softmax over destination nodes), 8 NeuronCores.

Strategy (graph partitioning by destination node, per the sharding hint):
- Host sorts edges by destination and bin-packs whole destination segments
  into 1024 partition-rows (8 devices x 128 rows x F slots). Every segment is
  fully contained in one partition-row of one device, so the segment softmax
  is entirely local: no collectives. The per-edge operand streams (src
  features and the destination feature of each edge) are laid out by the host
  in a feature-on-partitions [d, e] fp16 layout.
- Per device, per 1024-slot superblock:
    score^T = W_src^T @ src^T  (+)  W_dst^T @ dstf^T       (PSUM accumulate)
    a1 = Lrelu(score + b_src + b_dst)                      (evict to fp16)
    h  = W1^T @ a1                                         (PSUM)
    a2 = Lrelu(h + b1)                                     (fp16)
    logits-accumulation: col-tiled M=32 matmuls with a one-hot-padded w2;
      17 superblocks' logits land in distinct rows of one PSUM bank
      (other rows accumulate exact zeros), evicted once per block.
- Segment softmax via segmented scans on the [128, F] slot grid:
  forward scan accumulates per-segment sums, a reversed scan broadcasts each
  segment's total back to its slots, then attn = ex / total. exp() uses a
  constant shift instead of the per-segment max (cancels exactly in the
  softmax ratio; logits are O(+-5) so fp32 exp is safe).
"""

import os
import numpy as np

import concourse.bass as bass
import concourse.mybir as mybir
import concourse.tile as tile
from concourse import bacc
from concourse.bass_utils import run_bass_kernel_spmd

# ---------------- compile-time configuration ----------------
D = 128
CORES = 8
F = 1632                  # slots per partition row
EPAD = 128 * F            # 208896 slots per device
SB = 1024                 # superblock (slots) flowing through PSUM together
NSB = EPAD // SB          # 204
TILE = 512                # matmul moving free dim
LGB = 17                  # superblocks whose logits accumulate into one PSUM bank
NLGB = NSB // LGB         # 12 logit blocks
SHIFT = 16.0              # exp() stability shift (cancels in the softmax)
N_NODES = 50000
N_EDGES = 1600000

f32 = mybir.dt.float32
f16 = mybir.dt.float16

Lrelu = mybir.ActivationFunctionType.Lrelu
Exp = mybir.ActivationFunctionType.Exp
Copy = mybir.ActivationFunctionType.Copy
ADD = mybir.AluOpType.add
MULT = mybir.AluOpType.mult
MAX = mybir.AluOpType.max


def _a2_on_act(sb):
    """a1 evictions always run on ACT (they gate mm2, the loop-critical
    path). a2 evictions (which only gate the laggable logits accumulation)
    go mostly to DVE; ACT absorbs ~3/8 of them for balance."""
    return sb % 8 in (0, 3, 6)


def build_nc():
    nc = bacc.Bacc("TRN2", target_bir_lowering=False, debug=False)

    srcT_d = nc.dram_tensor("srcT", [128, EPAD], f16, kind="ExternalInput")
    dstT_d = nc.dram_tensor("dstT", [128, EPAD], f16, kind="ExternalInput")
    flags_d = nc.dram_tensor("flags", [128, F], f32, kind="ExternalInput")
    endm_d = nc.dram_tensor("endm", [128, F], f32, kind="ExternalInput")
    fbwd_d = nc.dram_tensor("fbwd", [128, F], f32, kind="ExternalInput")
    wsrc_d = nc.dram_tensor("wsrc", [D, D], f16, kind="ExternalInput")
    wdst_d = nc.dram_tensor("wdst", [D, D], f16, kind="ExternalInput")
    w1_d = nc.dram_tensor("w1", [D, D], f16, kind="ExternalInput")
    w2pad_d = nc.dram_tensor("w2pad", [D, LGB * 32], f16, kind="ExternalInput")
    bsum_d = nc.dram_tensor("bsum", [D, 1], f32, kind="ExternalInput")
    b1_d = nc.dram_tensor("b1", [D, 1], f32, kind="ExternalInput")
    bexp_d = nc.dram_tensor("bexp", [D, 1], f32, kind="ExternalInput")

    out_d = nc.dram_tensor("out", [128, F], f32, kind="ExternalOutput")
    lg_d = nc.dram_tensor("lg_scratch", [EPAD], f32)  # internal DRAM staging

    TPS = SB // TILE  # tiles per superblock (2)

    with tile.TileContext(nc) as tc:
        with tc.tile_pool(name="const", bufs=1) as cst:
            wsrc_s = cst.tile([D, D], f16)
            wdst_s = cst.tile([D, D], f16)
            w1_s = cst.tile([D, D], f16)
            w2pad_s = cst.tile([D, LGB * 32], f16)
            bsum_s = cst.tile([D, 1], f32)
            b1_s = cst.tile([D, 1], f32)
            bexp_s = cst.tile([D, 1], f32)
            for s, d in [(wsrc_s, wsrc_d), (wdst_s, wdst_d), (w1_s, w1_d),
                         (w2pad_s, w2pad_d), (bsum_s, bsum_d), (b1_s, b1_d),
                         (bexp_s, bexp_d)]:
                nc.sync.dma_start(s[:], d[:])

            # ---------------- phase 1: per-edge MLP -> logits ----------------
            with tc.tile_pool(name="stream", bufs=3) as stp, \
                 tc.tile_pool(name="act", bufs=4) as actp, \
                 tc.tile_pool(name="lgst", bufs=2) as lgstp, \
                 tc.tile_pool(name="ps", bufs=3, space="PSUM") as psp, \
                 tc.tile_pool(name="pslg", bufs=2, space="PSUM") as pslgp:
                lgp = None
                st4 = dt4 = None
                for sb in range(NSB):
                    if sb % 4 == 0:
                        o4 = sb * SB
                        st4 = stp.tile([128, 4 * SB], f16, tag="st4")
                        nc.sync.dma_start(st4[:], srcT_d[:, o4 : o4 + 4 * SB])
                        dt4 = stp.tile([128, 4 * SB], f16, tag="dt4")
                        nc.sync.dma_start(dt4[:], dstT_d[:, o4 : o4 + 4 * SB])
                    st = st4[:, (sb % 4) * SB : (sb % 4 + 1) * SB]
                    dt = dt4[:, (sb % 4) * SB : (sb % 4 + 1) * SB]

                    score = psp.tile([128, SB], f32, tag="ps")
                    for t in range(TPS):
                        nc.tensor.matmul(
                            score[:, t * TILE : (t + 1) * TILE],
                            wsrc_s[:], st[:, t * TILE : (t + 1) * TILE],
                            start=True, stop=False)
                    for t in range(TPS):
                        nc.tensor.matmul(
                            score[:, t * TILE : (t + 1) * TILE],
                            wdst_s[:], dt[:, t * TILE : (t + 1) * TILE],
                            start=False, stop=True)

                    a1 = actp.tile([128, SB], f16, tag="a1")
                    nc.scalar.activation(a1[:], score[:], Lrelu,
                                         bias=bsum_s[:], scale=1.0, alpha=0.01)

                    h = psp.tile([128, SB], f32, tag="ps")
                    for t in range(TPS):
                        nc.tensor.matmul(
                            h[:, t * TILE : (t + 1) * TILE],
                            w1_s[:], a1[:, t * TILE : (t + 1) * TILE],
                            start=True, stop=True)

                    a2 = actp.tile([128, SB], f16, tag="a2")
                    if _a2_on_act(sb):
                        nc.scalar.activation(a2[:], h[:], Lrelu,
                                             bias=b1_s[:], scale=1.0, alpha=0.01)
                    else:
                        nc.vector.tensor_scalar(a2[:], h[:], b1_s[:], None, ADD)
                        nc.vector.scalar_tensor_tensor(a2[:], a2[:], 0.01, a2[:], MULT, MAX)

                    # logits: col-tiled accumulating matmuls, M=32 with a
                    # one-hot-padded w2 (column k = sb % LGB). Row 32t+k of
                    # the shared bank receives this superblock's tile-t
                    # logits; all other rows accumulate exact zeros.
                    k = sb % LGB
                    if k == 0:
                        lgp = pslgp.tile([128, TILE], f32, tag="lg")
                    for t in range(TPS):
                        nc.tensor.matmul(
                            lgp[32 * t : 32 * t + 32, :],
                            w2pad_s[:, 32 * k : 32 * (k + 1)],
                            a2[:, t * TILE : (t + 1) * TILE],
                            start=(k == 0), stop=(k == LGB - 1),
                            tile_position=(0, 32 * t))
                    if k == LGB - 1:
                        blk = sb // LGB
                        lgs = lgstp.tile([64, TILE], f32, tag="lgs")
                        nc.scalar.activation(lgs[:], lgp[0:64, :], Copy)
                        # superblock j of this block, tile t lives at
                        # lgs[32*t + j, :] -> lg_d[(blk*LGB+j)*SB + t*TILE :+TILE]
                        lgv = lg_d[:].rearrange("(j t f) -> j t f", t=TPS, f=TILE)
                        for t in range(TPS):
                            nc.sync.dma_start(
                                lgv[blk * LGB : (blk + 1) * LGB, t, :],
                                lgs[32 * t : 32 * t + LGB, :])

            # ---------------- phase 2: segment softmax ----------------
            with tc.tile_pool(name="soft", bufs=1) as sfp:
                lgsc = sfp.tile([128, F], f32)
                nc.sync.dma_start(lgsc[:], lg_d[:].rearrange("(p f) -> p f", p=128))
                flags_s = sfp.tile([128, F], f32)
                endm_s = sfp.tile([128, F], f32)
                fbwd_s = sfp.tile([128, F], f32)
                nc.sync.dma_start(flags_s[:], flags_d[:])
                nc.sync.dma_start(endm_s[:], endm_d[:])
                nc.sync.dma_start(fbwd_s[:], fbwd_d[:])

                ex = sfp.tile([128, F], f32)
                nc.scalar.activation(ex[:], lgsc[:], Exp, bias=bexp_s[:], scale=1.0)

                S = sfp.tile([128, F], f32)
                nc.vector.tensor_tensor_scan(S[:], flags_s[:], ex[:], 0.0, MULT, ADD)
                dend = sfp.tile([128, F], f32)
                nc.vector.tensor_tensor(dend[:], S[:], endm_s[:], MULT)
                Trev = sfp.tile([128, F], f32)
                nc.vector.tensor_tensor_scan(Trev[:], fbwd_s[:], dend[:, ::-1], 0.0, MULT, ADD)
                R = sfp.tile([128, F], f32)
                nc.vector.reciprocal(R[:], Trev[:])
                attn = sfp.tile([128, F], f32)
                nc.vector.tensor_tensor(attn[:], ex[:], R[:, ::-1], MULT)
                nc.sync.dma_start(out_d[:], attn[:])

    nc.finalize()
    return nc


# ---------------- host-side packing ----------------

def _pack(edge_dst):
    order = np.argsort(edge_dst, kind="stable")
    sdst = edge_dst[order].astype(np.int64)
    counts = np.bincount(edge_dst, minlength=N_NODES).astype(np.int64)

    row_of_node = np.empty(N_NODES, np.int64)
    col_of_node = np.empty(N_NODES, np.int64)
    row, col = 0, 0
    for n in range(N_NODES):
        c = counts[n]
        if col + c > F:
            row += 1
            col = 0
        row_of_node[n] = row
        col_of_node[n] = col
        col += c
    assert row < 128 * CORES, f"packing overflow: {row}"

    starts = np.cumsum(counts) - counts
    within = np.arange(N_EDGES, dtype=np.int64) - starts[sdst]
    slot_global = row_of_node[sdst] * F + col_of_node[sdst] + within
    dev_of_edge = (row_of_node[sdst] // 128).astype(np.int64)
    slot_in_dev = slot_global - dev_of_edge * EPAD
    return dict(order=order, sdst=sdst, dev_of_edge=dev_of_edge,
                slot_in_dev=slot_in_dev)


def _device_inputs(P, src, dstf, edge_dst, d):
    mask = P["dev_of_edge"] == d
    slots = P["slot_in_dev"][mask]
    eids = P["order"][mask]

    srcT = np.zeros((EPAD, D), np.float16)
    srcT[slots] = src[eids].astype(np.float16)
    srcT = np.ascontiguousarray(srcT.T)

    dstT = np.zeros((EPAD, D), np.float16)
    dstT[slots] = dstf[edge_dst[eids]].astype(np.float16)
    dstT = np.ascontiguousarray(dstT.T)

    used = np.zeros(EPAD, bool)
    used[slots] = True
    fl = np.ones(EPAD, np.float32)
    sd = P["sdst"][mask]
    seg_start_slots = slots[np.concatenate([[True], sd[1:] != sd[:-1]])]
    fl[seg_start_slots] = 0.0
    prev_used = np.concatenate([[False], used[:-1]])
    run_start = (~used) & (prev_used | (np.arange(EPAD) % F == 0))
    fl[run_start] = 0.0
    fl[np.arange(0, EPAD, F)] = 0.0
    flags = fl.reshape(128, F)

    nxt_reset = np.concatenate([flags[:, 1:], np.zeros((128, 1), np.float32)], axis=1)
    endm = np.where(nxt_reset == 0.0, 1.0, 0.0).astype(np.float32)
    fbwd = np.ascontiguousarray((1.0 - endm)[:, ::-1])

    return dict(srcT=srcT, dstT=dstT, flags=flags, endm=endm, fbwd=fbwd), slots, eids


_CACHE = {}


def run(inputs, trace=False):
    src = np.asarray(inputs["src_feat"], np.float32)
    dstf = np.asarray(inputs["dst_feat"], np.float32)
    edge_dst = np.asarray(inputs["edge_dst"]).astype(np.int64)
    assert src.shape == (N_EDGES, D) and dstf.shape == (N_NODES, D)

    P = _pack(edge_dst)

    wsrc = np.asarray(inputs["W_src"], np.float32).astype(np.float16)
    wdst = np.asarray(inputs["W_dst"], np.float32).astype(np.float16)
    w1 = np.asarray(inputs["W1"], np.float32).astype(np.float16)
    w2v = np.asarray(inputs["W2"], np.float32).astype(np.float16).reshape(D)
    w2pad = np.zeros((D, LGB * 32), np.float16)
    for k in range(LGB):
        w2pad[:, 32 * k + k] = w2v
    bsum = (np.asarray(inputs["b_src"], np.float32)
            + np.asarray(inputs["b_dst"], np.float32)).reshape(D, 1)
    b1 = np.asarray(inputs["b1"], np.float32).reshape(D, 1)
    bexp = np.full((D, 1), float(np.asarray(inputs["b2"]).reshape(-1)[0]) - SHIFT,
                   np.float32)

    in_maps = []
    recov = []
    for d in range(CORES):
        dv, slots, eids = _device_inputs(P, src, dstf, edge_dst, d)
        dv.update(wsrc=wsrc, wdst=wdst, w1=w1, w2pad=w2pad, bsum=bsum, b1=b1,
                  bexp=bexp)
        in_maps.append(dv)
        recov.append((slots, eids))

    if "nc" not in _CACHE:
        _CACHE["nc"] = build_nc()
    nc = _CACHE["nc"]

    res = run_bass_kernel_spmd(nc, in_maps, list(range(CORES)), trace=trace)

    out = np.empty(N_EDGES, np.float32)
    for d in range(CORES):
        slots, eids = recov[d]
        vals = np.asarray(res.results[d]["out"], np.float32).reshape(-1)
        out[eids] = vals[slots]
    _CACHE["exec_time_ns"] = res.exec_time_ns
    _CACHE["trace_path"] = (res.instructions_and_trace or (None, None))[1]
    return out[:, None]


def kernel(**inputs):
    return run(inputs, trace=bool(os.environ.get("BASS_TRACE")))


# revision 20
# speedup vs baseline: 1.3252x; 1.1587x over previous
"""Trainium2 Bass kernel for nn_AttentionWithEpinions (GNN edge attention with
segment softmax over destination nodes), 8 NeuronCores.

Strategy (graph partitioning by destination node, per the sharding hint):
- Host sorts edges by destination and bin-packs whole destination segments
  into 1024 partition-rows (8 devices x 128 rows x F slots). Every segment is
  fully contained in one partition-row of one device, so the segment softmax
  is entirely local: no collectives. The per-edge operand streams (src
  features and the destination feature of each edge) are laid out by the host
  in a feature-on-partitions [d, e] fp16 layout.
- Per device, per 1024-slot superblock:
    score^T = W_src^T @ src^T  (+)  W_dst^T @ dstf^T       (PSUM accumulate)
    a1 = Lrelu(score + b_src + b_dst)                      (evict to fp16)
    h  = W1^T @ a1                                         (PSUM)
    a2 = Lrelu(h + b1)                                     (fp16)
    logits-accumulation: col-tiled M=32 matmuls with a one-hot-padded w2;
      17 superblocks' logits land in distinct rows of one PSUM bank
      (other rows accumulate exact zeros), evicted once per block.
- Segment softmax via segmented scans on the [128, F] slot grid:
  forward scan accumulates per-segment sums, a reversed scan broadcasts each
  segment's total back to its slots, then attn = ex / total. exp() uses a
  constant shift instead of the per-segment max (cancels exactly in the
  softmax ratio; logits are O(+-5) so fp32 exp is safe).
"""

import os
import numpy as np

import concourse.bass as bass
import concourse.mybir as mybir
import concourse.tile as tile
from concourse import bacc
from concourse.bass_utils import run_bass_kernel_spmd

# ---------------- compile-time configuration ----------------
D = 128
CORES = 8
F = 1632                  # slots per partition row
EPAD = 128 * F            # 208896 slots per device
SB = 1024                 # superblock (slots) flowing through PSUM together
NSB = EPAD // SB          # 204
TILE = 512                # matmul moving free dim
LGB = 17                  # superblocks whose logits accumulate into one PSUM bank
NLGB = NSB // LGB         # 12 logit blocks
SHIFT = 16.0              # exp() stability shift (cancels in the softmax)
N_NODES = 50000
N_EDGES = 1600000

f32 = mybir.dt.float32
f16 = mybir.dt.float16

Lrelu = mybir.ActivationFunctionType.Lrelu
Exp = mybir.ActivationFunctionType.Exp
Copy = mybir.ActivationFunctionType.Copy
ADD = mybir.AluOpType.add
MULT = mybir.AluOpType.mult
MAX = mybir.AluOpType.max


def _a2_on_act(sb):
    """a1 evictions always run on ACT (they gate mm2, the loop-critical
    path). a2 evictions (which only gate the laggable logits accumulation)
    go mostly to DVE; ACT absorbs ~3/8 of them for balance."""
    return sb % 8 in (0, 3, 6)


def build_nc():
    nc = bacc.Bacc("TRN2", target_bir_lowering=False, debug=False)

    srcT_d = nc.dram_tensor("srcT", [128, EPAD], f16, kind="ExternalInput")
    dstT_d = nc.dram_tensor("dstT", [128, EPAD], f16, kind="ExternalInput")
    flags_d = nc.dram_tensor("flags", [128, F], f32, kind="ExternalInput")
    endm_d = nc.dram_tensor("endm", [128, F], f32, kind="ExternalInput")
    fbwd_d = nc.dram_tensor("fbwd", [128, F], f32, kind="ExternalInput")
    wsrc_d = nc.dram_tensor("wsrc", [D, D], f16, kind="ExternalInput")
    wdst_d = nc.dram_tensor("wdst", [D, D], f16, kind="ExternalInput")
    w1_d = nc.dram_tensor("w1", [D, D], f16, kind="ExternalInput")
    w2pad_d = nc.dram_tensor("w2pad", [D, LGB * 32], f16, kind="ExternalInput")
    bsum_d = nc.dram_tensor("bsum", [D, 1], f32, kind="ExternalInput")
    b1_d = nc.dram_tensor("b1", [D, 1], f32, kind="ExternalInput")
    bexp_d = nc.dram_tensor("bexp", [D, 1], f32, kind="ExternalInput")

    out_d = nc.dram_tensor("out", [128, F], f32, kind="ExternalOutput")
    lg_d = nc.dram_tensor("lg_scratch", [EPAD], f32)  # internal DRAM staging

    TPS = SB // TILE  # tiles per superblock (2)

    with tile.TileContext(nc) as tc:
        with tc.tile_pool(name="const", bufs=1) as cst:
            wsrc_s = cst.tile([D, D], f16)
            wdst_s = cst.tile([D, D], f16)
            w1_s = cst.tile([D, D], f16)
            w2pad_s = cst.tile([D, LGB * 32], f16)
            bsum_s = cst.tile([D, 1], f32)
            b1_s = cst.tile([D, 1], f32)
            bexp_s = cst.tile([D, 1], f32)
            for s, d in [(wsrc_s, wsrc_d), (wdst_s, wdst_d), (w1_s, w1_d),
                         (w2pad_s, w2pad_d), (bsum_s, bsum_d), (b1_s, b1_d),
                         (bexp_s, bexp_d)]:
                nc.sync.dma_start(s[:], d[:])

            # ---------------- phase 1: per-edge MLP -> logits ----------------
            with tc.tile_pool(name="stream", bufs=3) as stp, \
                 tc.tile_pool(name="act", bufs=4) as actp, \
                 tc.tile_pool(name="lgst", bufs=2) as lgstp, \
                 tc.tile_pool(name="ps", bufs=3, space="PSUM") as psp, \
                 tc.tile_pool(name="pslg", bufs=2, space="PSUM") as pslgp:
                lgp = None
                st4 = dt4 = None
                for sb in range(NSB):
                    if sb % 4 == 0:
                        o4 = sb * SB
                        st4 = stp.tile([128, 4 * SB], f16, tag="st4")
                        nc.sync.dma_start(st4[:], srcT_d[:, o4 : o4 + 4 * SB])
                        dt4 = stp.tile([128, 4 * SB], f16, tag="dt4")
                        nc.sync.dma_start(dt4[:], dstT_d[:, o4 : o4 + 4 * SB])
                    st = st4[:, (sb % 4) * SB : (sb % 4 + 1) * SB]
                    dt = dt4[:, (sb % 4) * SB : (sb % 4 + 1) * SB]

                    score = psp.tile([128, SB], f32, tag="ps")
                    for t in range(TPS):
                        nc.tensor.matmul(
                            score[:, t * TILE : (t + 1) * TILE],
                            wsrc_s[:], st[:, t * TILE : (t + 1) * TILE],
                            start=True, stop=False)
                    for t in range(TPS):
                        nc.tensor.matmul(
                            score[:, t * TILE : (t + 1) * TILE],
                            wdst_s[:], dt[:, t * TILE : (t + 1) * TILE],
                            start=False, stop=True)

                    a1 = actp.tile([128, SB], f16, tag="a1")
                    for t in range(TPS):
                        nc.scalar.activation(a1[:, t * TILE : (t + 1) * TILE],
                                             score[:, t * TILE : (t + 1) * TILE],
                                             Lrelu, bias=bsum_s[:], scale=1.0,
                                             alpha=0.01)

                    h = psp.tile([128, SB], f32, tag="ps")
                    for t in range(TPS):
                        nc.tensor.matmul(
                            h[:, t * TILE : (t + 1) * TILE],
                            w1_s[:], a1[:, t * TILE : (t + 1) * TILE],
                            start=True, stop=True)

                    a2 = actp.tile([128, SB], f16, tag="a2")
                    if _a2_on_act(sb):
                        nc.scalar.activation(a2[:], h[:], Lrelu,
                                             bias=b1_s[:], scale=1.0, alpha=0.01)
                    else:
                        a2t = actp.tile([128, SB], f16, tag="a2t")
                        nc.vector.tensor_scalar(a2t[:], h[:], b1_s[:], None, ADD)
                        nc.vector.scalar_tensor_tensor(a2[:], a2t[:], 0.01, a2t[:], MULT, MAX)

                    # logits: col-tiled accumulating matmuls, M=32 with a
                    # one-hot-padded w2 (column k = sb % LGB). Row 32t+k of
                    # the shared bank receives this superblock's tile-t
                    # logits; all other rows accumulate exact zeros.
                    k = sb % LGB
                    if k == 0:
                        lgp = pslgp.tile([128, TILE], f32, tag="lg")
                    for t in range(TPS):
                        nc.tensor.matmul(
                            lgp[32 * t : 32 * t + 32, :],
                            w2pad_s[:, 32 * k : 32 * (k + 1)],
                            a2[:, t * TILE : (t + 1) * TILE],
                            start=(k == 0), stop=(k == LGB - 1),
                            tile_position=(0, 32 * t))
                    if k == LGB - 1:
                        blk = sb // LGB
                        lgs = lgstp.tile([64, TILE], f32, tag="lgs")
                        nc.scalar.activation(lgs[:], lgp[0:64, :], Copy)
                        # superblock j of this block, tile t lives at
                        # lgs[32*t + j, :] -> lg_d[(blk*LGB+j)*SB + t*TILE :+TILE]
                        lgv = lg_d[:].rearrange("(j t f) -> j t f", t=TPS, f=TILE)
                        for t in range(TPS):
                            nc.sync.dma_start(
                                lgv[blk * LGB : (blk + 1) * LGB, t, :],
                                lgs[32 * t : 32 * t + LGB, :])

            # ---------------- phase 2: segment softmax ----------------
            with tc.tile_pool(name="soft", bufs=1) as sfp:
                lgsc = sfp.tile([128, F], f32)
                nc.sync.dma_start(lgsc[:], lg_d[:].rearrange("(p f) -> p f", p=128))
                flags_s = sfp.tile([128, F], f32)
                endm_s = sfp.tile([128, F], f32)
                fbwd_s = sfp.tile([128, F], f32)
                nc.sync.dma_start(flags_s[:], flags_d[:])
                nc.sync.dma_start(endm_s[:], endm_d[:])
                nc.sync.dma_start(fbwd_s[:], fbwd_d[:])

                ex = sfp.tile([128, F], f32)
                nc.scalar.activation(ex[:], lgsc[:], Exp, bias=bexp_s[:], scale=1.0)

                S = sfp.tile([128, F], f32)
                nc.vector.tensor_tensor_scan(S[:], flags_s[:], ex[:], 0.0, MULT, ADD)
                dend = sfp.tile([128, F], f32)
                nc.vector.tensor_tensor(dend[:], S[:], endm_s[:], MULT)
                Trev = sfp.tile([128, F], f32)
                nc.vector.tensor_tensor_scan(Trev[:], fbwd_s[:], dend[:, ::-1], 0.0, MULT, ADD)
                R = sfp.tile([128, F], f32)
                nc.vector.reciprocal(R[:], Trev[:])
                attn = sfp.tile([128, F], f32)
                nc.vector.tensor_tensor(attn[:], ex[:], R[:, ::-1], MULT)
                nc.sync.dma_start(out_d[:], attn[:])

    nc.finalize()
    return nc


# ---------------- host-side packing ----------------

def _pack(edge_dst):
    order = np.argsort(edge_dst, kind="stable")
    sdst = edge_dst[order].astype(np.int64)
    counts = np.bincount(edge_dst, minlength=N_NODES).astype(np.int64)

    row_of_node = np.empty(N_NODES, np.int64)
    col_of_node = np.empty(N_NODES, np.int64)
    row, col = 0, 0
    for n in range(N_NODES):
        c = counts[n]
        if col + c > F:
            row += 1
            col = 0
        row_of_node[n] = row
        col_of_node[n] = col
        col += c
    assert row < 128 * CORES, f"packing overflow: {row}"

    starts = np.cumsum(counts) - counts
    within = np.arange(N_EDGES, dtype=np.int64) - starts[sdst]
    slot_global = row_of_node[sdst] * F + col_of_node[sdst] + within
    dev_of_edge = (row_of_node[sdst] // 128).astype(np.int64)
    slot_in_dev = slot_global - dev_of_edge * EPAD
    return dict(order=order, sdst=sdst, dev_of_edge=dev_of_edge,
                slot_in_dev=slot_in_dev)


def _device_inputs(P, src, dstf, edge_dst, d):
    mask = P["dev_of_edge"] == d
    slots = P["slot_in_dev"][mask]
    eids = P["order"][mask]

    srcT = np.zeros((EPAD, D), np.float16)
    srcT[slots] = src[eids].astype(np.float16)
    srcT = np.ascontiguousarray(srcT.T)

    dstT = np.zeros((EPAD, D), np.float16)
    dstT[slots] = dstf[edge_dst[eids]].astype(np.float16)
    dstT = np.ascontiguousarray(dstT.T)

    used = np.zeros(EPAD, bool)
    used[slots] = True
    fl = np.ones(EPAD, np.float32)
    sd = P["sdst"][mask]
    seg_start_slots = slots[np.concatenate([[True], sd[1:] != sd[:-1]])]
    fl[seg_start_slots] = 0.0
    prev_used = np.concatenate([[False], used[:-1]])
    run_start = (~used) & (prev_used | (np.arange(EPAD) % F == 0))
    fl[run_start] = 0.0
    fl[np.arange(0, EPAD, F)] = 0.0
    flags = fl.reshape(128, F)

    nxt_reset = np.concatenate([flags[:, 1:], np.zeros((128, 1), np.float32)], axis=1)
    endm = np.where(nxt_reset == 0.0, 1.0, 0.0).astype(np.float32)
    fbwd = np.ascontiguousarray((1.0 - endm)[:, ::-1])

    return dict(srcT=srcT, dstT=dstT, flags=flags, endm=endm, fbwd=fbwd), slots, eids


_CACHE = {}


def run(inputs, trace=False):
    src = np.asarray(inputs["src_feat"], np.float32)
    dstf = np.asarray(inputs["dst_feat"], np.float32)
    edge_dst = np.asarray(inputs["edge_dst"]).astype(np.int64)
    assert src.shape == (N_EDGES, D) and dstf.shape == (N_NODES, D)

    P = _pack(edge_dst)

    wsrc = np.asarray(inputs["W_src"], np.float32).astype(np.float16)
    wdst = np.asarray(inputs["W_dst"], np.float32).astype(np.float16)
    w1 = np.asarray(inputs["W1"], np.float32).astype(np.float16)
    w2v = np.asarray(inputs["W2"], np.float32).astype(np.float16).reshape(D)
    w2pad = np.zeros((D, LGB * 32), np.float16)
    for k in range(LGB):
        w2pad[:, 32 * k + k] = w2v
    bsum = (np.asarray(inputs["b_src"], np.float32)
            + np.asarray(inputs["b_dst"], np.float32)).reshape(D, 1)
    b1 = np.asarray(inputs["b1"], np.float32).reshape(D, 1)
    bexp = np.full((D, 1), float(np.asarray(inputs["b2"]).reshape(-1)[0]) - SHIFT,
                   np.float32)

    in_maps = []
    recov = []
    for d in range(CORES):
        dv, slots, eids = _device_inputs(P, src, dstf, edge_dst, d)
        dv.update(wsrc=wsrc, wdst=wdst, w1=w1, w2pad=w2pad, bsum=bsum, b1=b1,
                  bexp=bexp)
        in_maps.append(dv)
        recov.append((slots, eids))

    if "nc" not in _CACHE:
        _CACHE["nc"] = build_nc()
    nc = _CACHE["nc"]

    res = run_bass_kernel_spmd(nc, in_maps, list(range(CORES)), trace=trace)

    out = np.empty(N_EDGES, np.float32)
    for d in range(CORES):
        slots, eids = recov[d]
        vals = np.asarray(res.results[d]["out"], np.float32).reshape(-1)
        out[eids] = vals[slots]
    _CACHE["exec_time_ns"] = res.exec_time_ns
    _CACHE["trace_path"] = (res.instructions_and_trace or (None, None))[1]
    return out[:, None]


def kernel(**inputs):
    return run(inputs, trace=bool(os.environ.get("BASS_TRACE")))


# revision 23
# speedup vs baseline: 1.4712x; 1.1102x over previous
"""Trainium2 Bass kernel for nn_AttentionWithEpinions (GNN edge attention with
segment softmax over destination nodes), 8 NeuronCores.

Strategy (graph partitioning by destination node, per the sharding hint):
- Host sorts edges by destination and bin-packs whole destination segments
  into 1024 partition-rows (8 devices x 128 rows x F slots). Every segment is
  fully contained in one partition-row of one device, so the segment softmax
  is entirely local: no collectives. The per-edge operand streams (src
  features and the destination feature of each edge) are laid out by the host
  in a feature-on-partitions [d, e] fp16 layout.
- Per device, per 1024-slot superblock:
    score^T = W_src^T @ src^T  (+)  W_dst^T @ dstf^T       (PSUM accumulate)
    a1 = Lrelu(score + b_src + b_dst)                      (evict to fp16)
    h  = W1^T @ a1                                         (PSUM)
    a2 = Lrelu(h + b1)                                     (fp16)
    logits-accumulation: col-tiled M=32 matmuls with a one-hot-padded w2;
      17 superblocks' logits land in distinct rows of one PSUM bank
      (other rows accumulate exact zeros), evicted once per block.
- Segment softmax via segmented scans on the [128, F] slot grid:
  forward scan accumulates per-segment sums, a reversed scan broadcasts each
  segment's total back to its slots, then attn = ex / total. exp() uses a
  constant shift instead of the per-segment max (cancels exactly in the
  softmax ratio; logits are O(+-5) so fp32 exp is safe).
"""

import os
import numpy as np

import concourse.bass as bass
import concourse.mybir as mybir
import concourse.tile as tile
from concourse import bacc
from concourse.bass_utils import run_bass_kernel_spmd

# ---------------- compile-time configuration ----------------
D = 128
CORES = 8
F = 1632                  # slots per partition row
EPAD = 128 * F            # 208896 slots per device
SB = 1024                 # superblock (slots) flowing through PSUM together
NSB = EPAD // SB          # 204
TILE = 512                # matmul moving free dim
LGB = 17                  # superblocks whose logits accumulate into one PSUM bank
NLGB = NSB // LGB         # 12 logit blocks
SHIFT = 16.0              # exp() stability shift (cancels in the softmax)
N_NODES = 50000
N_EDGES = 1600000

f32 = mybir.dt.float32
f16 = mybir.dt.float16

Lrelu = mybir.ActivationFunctionType.Lrelu
Exp = mybir.ActivationFunctionType.Exp
Copy = mybir.ActivationFunctionType.Copy
ADD = mybir.AluOpType.add
MULT = mybir.AluOpType.mult
MAX = mybir.AluOpType.max


def _a2_on_act(sb):
    """a1 evictions always run on ACT (they gate mm2, the loop-critical
    path). a2 evictions (which only gate the laggable logits accumulation)
    go mostly to DVE; ACT absorbs ~3/8 of them for balance."""
    return sb % 8 in (0, 3, 6)


def build_nc():
    nc = bacc.Bacc("TRN2", target_bir_lowering=False, debug=False)

    srcT_d = nc.dram_tensor("srcT", [128, EPAD], f16, kind="ExternalInput")
    dstT_d = nc.dram_tensor("dstT", [128, EPAD], f16, kind="ExternalInput")
    flags_d = nc.dram_tensor("flags", [128, F], f32, kind="ExternalInput")
    endm_d = nc.dram_tensor("endm", [128, F], f32, kind="ExternalInput")
    fbwd_d = nc.dram_tensor("fbwd", [128, F], f32, kind="ExternalInput")
    wsrc_d = nc.dram_tensor("wsrc", [D, D], f16, kind="ExternalInput")
    wdst_d = nc.dram_tensor("wdst", [D, D], f16, kind="ExternalInput")
    w1_d = nc.dram_tensor("w1", [D, D], f16, kind="ExternalInput")
    w2pad_d = nc.dram_tensor("w2pad", [D, LGB * 32], f16, kind="ExternalInput")
    bsum_d = nc.dram_tensor("bsum", [D, 1], f32, kind="ExternalInput")
    b1_d = nc.dram_tensor("b1", [D, 1], f32, kind="ExternalInput")
    bexp_d = nc.dram_tensor("bexp", [D, 1], f32, kind="ExternalInput")

    out_d = nc.dram_tensor("out", [128, F], f32, kind="ExternalOutput")
    lg_d = nc.dram_tensor("lg_scratch", [EPAD], f32)  # internal DRAM staging

    TPS = SB // TILE  # tiles per superblock (2)

    with tile.TileContext(nc) as tc:
        with tc.tile_pool(name="const", bufs=1) as cst:
            wsrc_s = cst.tile([D, D], f16)
            wdst_s = cst.tile([D, D], f16)
            w1_s = cst.tile([D, D], f16)
            w2pad_s = cst.tile([D, LGB * 32], f16)
            bsum_s = cst.tile([D, 1], f32)
            b1_s = cst.tile([D, 1], f32)
            bexp_s = cst.tile([D, 1], f32)
            for s, d in [(wsrc_s, wsrc_d), (wdst_s, wdst_d), (w1_s, w1_d),
                         (w2pad_s, w2pad_d), (bsum_s, bsum_d), (b1_s, b1_d),
                         (bexp_s, bexp_d)]:
                nc.sync.dma_start(s[:], d[:])

            # ---------------- phase 1: per-edge MLP -> logits ----------------
            with tc.tile_pool(name="stream", bufs=3) as stp, \
                 tc.tile_pool(name="act", bufs=4) as actp, \
                 tc.tile_pool(name="lgst", bufs=2) as lgstp, \
                 tc.tile_pool(name="ps", bufs=3, space="PSUM") as psp, \
                 tc.tile_pool(name="pslg", bufs=2, space="PSUM") as pslgp:
                lgp = None
                st4 = dt4 = None
                PAIR_BLK = 2 * LGB  # 34 superblocks accumulate per logits bank
                for pi in range(NSB // 2):
                    sb0 = 2 * pi
                    if sb0 % 4 == 0:
                        o4 = sb0 * SB
                        st4 = stp.tile([128, 4 * SB], f16, tag="st4")
                        nc.sync.dma_start(st4[:], srcT_d[:, o4 : o4 + 4 * SB])
                        dt4 = stp.tile([128, 4 * SB], f16, tag="dt4")
                        nc.sync.dma_start(dt4[:], dstT_d[:, o4 : o4 + 4 * SB])
                    q = (sb0 % 4) * SB
                    sts = [st4[:, q : q + SB], st4[:, q + SB : q + 2 * SB]]
                    dts = [dt4[:, q : q + SB], dt4[:, q + SB : q + 2 * SB]]

                    scores = [psp.tile([128, SB], f32, tag="ps", name=f"score{pi}_{i}") for i in range(2)]
                    # weight-phase grouped matmuls: all W_src, then all W_dst
                    for p in range(2):
                        for t in range(TPS):
                            nc.tensor.matmul(
                                scores[p][:, t * TILE : (t + 1) * TILE],
                                wsrc_s[:], sts[p][:, t * TILE : (t + 1) * TILE],
                                start=True, stop=False)
                    for p in range(2):
                        for t in range(TPS):
                            nc.tensor.matmul(
                                scores[p][:, t * TILE : (t + 1) * TILE],
                                wdst_s[:], dts[p][:, t * TILE : (t + 1) * TILE],
                                start=False, stop=True)

                    a1s = [actp.tile([128, SB], f16, tag="a1", name=f"a1_{pi}_{i}") for i in range(2)]
                    for p in range(2):
                        for t in range(TPS):
                            nc.scalar.activation(
                                a1s[p][:, t * TILE : (t + 1) * TILE],
                                scores[p][:, t * TILE : (t + 1) * TILE],
                                Lrelu, bias=bsum_s[:], scale=1.0, alpha=0.01)

                    hs = []
                    for p in range(2):
                        h = psp.tile([128, SB], f32, tag="ps", name=f"h{pi}_{p}")
                        hs.append(h)
                        for t in range(TPS):
                            nc.tensor.matmul(
                                h[:, t * TILE : (t + 1) * TILE],
                                w1_s[:], a1s[p][:, t * TILE : (t + 1) * TILE],
                                start=True, stop=True)

                    a2s = [actp.tile([128, SB], f16, tag="a2", name=f"a2_{pi}_{i}") for i in range(2)]
                    for p in range(2):
                        sb = sb0 + p
                        if sb % 10 in (0, 3, 7):
                            nc.scalar.activation(a2s[p][:], hs[p][:], Lrelu,
                                                 bias=b1_s[:], scale=1.0, alpha=0.01)
                        else:
                            a2t = actp.tile([128, SB], f16, tag="a2t", name=f"a2t_{pi}_{p}")
                            nc.vector.tensor_scalar(a2t[:], hs[p][:], b1_s[:], None, ADD)
                            nc.vector.scalar_tensor_tensor(a2s[p][:], a2t[:], 0.01, a2t[:], MULT, MAX)

                    # logits: 4-way col-tiled accumulating matmuls (M=32).
                    # Column group j = 2*t + p holds pair-member p, tile t;
                    # within a block, pair k of LGB lands on row 32*j + k.
                    k = pi % LGB
                    if k == 0:
                        lgp = pslgp.tile([128, TILE], f32, tag="lg")
                    for p in range(2):
                        for t in range(TPS):
                            j = 2 * t + p
                            nc.tensor.matmul(
                                lgp[32 * j : 32 * j + 32, :],
                                w2pad_s[:, 32 * k : 32 * (k + 1)],
                                a2s[p][:, t * TILE : (t + 1) * TILE],
                                start=(k == 0), stop=(k == LGB - 1),
                                tile_position=(0, 32 * j))
                    if k == LGB - 1:
                        blk = pi // LGB
                        lgs = lgstp.tile([128, TILE], f32, tag="lgs")
                        nc.scalar.activation(lgs[:], lgp[:], Copy)
                        # row 32*(2t+p)+k  ->  sb = blk*PAIR_BLK + 2k + p, tile t
                        lgv = lg_d[:].rearrange("(j t f) -> j t f", t=TPS, f=TILE)
                        for p in range(2):
                            for t in range(TPS):
                                j = 2 * t + p
                                nc.sync.dma_start(
                                    lgv[blk * PAIR_BLK + p : blk * PAIR_BLK + p + 2 * LGB - 1 : 2, t, :],
                                    lgs[32 * j : 32 * j + LGB, :])

            # ---------------- phase 2: segment softmax ----------------
            with tc.tile_pool(name="soft", bufs=1) as sfp:
                lgsc = sfp.tile([128, F], f32)
                nc.sync.dma_start(lgsc[:], lg_d[:].rearrange("(p f) -> p f", p=128))
                flags_s = sfp.tile([128, F], f32)
                endm_s = sfp.tile([128, F], f32)
                fbwd_s = sfp.tile([128, F], f32)
                nc.sync.dma_start(flags_s[:], flags_d[:])
                nc.sync.dma_start(endm_s[:], endm_d[:])
                nc.sync.dma_start(fbwd_s[:], fbwd_d[:])

                ex = sfp.tile([128, F], f32)
                nc.scalar.activation(ex[:], lgsc[:], Exp, bias=bexp_s[:], scale=1.0)

                S = sfp.tile([128, F], f32)
                nc.vector.tensor_tensor_scan(S[:], flags_s[:], ex[:], 0.0, MULT, ADD)
                dend = sfp.tile([128, F], f32)
                nc.vector.tensor_tensor(dend[:], S[:], endm_s[:], MULT)
                Trev = sfp.tile([128, F], f32)
                nc.vector.tensor_tensor_scan(Trev[:], fbwd_s[:], dend[:, ::-1], 0.0, MULT, ADD)
                R = sfp.tile([128, F], f32)
                nc.vector.reciprocal(R[:], Trev[:])
                attn = sfp.tile([128, F], f32)
                nc.vector.tensor_tensor(attn[:], ex[:], R[:, ::-1], MULT)
                nc.sync.dma_start(out_d[:], attn[:])

    nc.finalize()
    return nc


# ---------------- host-side packing ----------------

def _pack(edge_dst):
    order = np.argsort(edge_dst, kind="stable")
    sdst = edge_dst[order].astype(np.int64)
    counts = np.bincount(edge_dst, minlength=N_NODES).astype(np.int64)

    row_of_node = np.empty(N_NODES, np.int64)
    col_of_node = np.empty(N_NODES, np.int64)
    row, col = 0, 0
    for n in range(N_NODES):
        c = counts[n]
        if col + c > F:
            row += 1
            col = 0
        row_of_node[n] = row
        col_of_node[n] = col
        col += c
    assert row < 128 * CORES, f"packing overflow: {row}"

    starts = np.cumsum(counts) - counts
    within = np.arange(N_EDGES, dtype=np.int64) - starts[sdst]
    slot_global = row_of_node[sdst] * F + col_of_node[sdst] + within
    dev_of_edge = (row_of_node[sdst] // 128).astype(np.int64)
    slot_in_dev = slot_global - dev_of_edge * EPAD
    return dict(order=order, sdst=sdst, dev_of_edge=dev_of_edge,
                slot_in_dev=slot_in_dev)


def _device_inputs(P, src, dstf, edge_dst, d):
    mask = P["dev_of_edge"] == d
    slots = P["slot_in_dev"][mask]
    eids = P["order"][mask]

    srcT = np.zeros((EPAD, D), np.float16)
    srcT[slots] = src[eids].astype(np.float16)
    srcT = np.ascontiguousarray(srcT.T)

    dstT = np.zeros((EPAD, D), np.float16)
    dstT[slots] = dstf[edge_dst[eids]].astype(np.float16)
    dstT = np.ascontiguousarray(dstT.T)

    used = np.zeros(EPAD, bool)
    used[slots] = True
    fl = np.ones(EPAD, np.float32)
    sd = P["sdst"][mask]
    seg_start_slots = slots[np.concatenate([[True], sd[1:] != sd[:-1]])]
    fl[seg_start_slots] = 0.0
    prev_used = np.concatenate([[False], used[:-1]])
    run_start = (~used) & (prev_used | (np.arange(EPAD) % F == 0))
    fl[run_start] = 0.0
    fl[np.arange(0, EPAD, F)] = 0.0
    flags = fl.reshape(128, F)

    nxt_reset = np.concatenate([flags[:, 1:], np.zeros((128, 1), np.float32)], axis=1)
    endm = np.where(nxt_reset == 0.0, 1.0, 0.0).astype(np.float32)
    fbwd = np.ascontiguousarray((1.0 - endm)[:, ::-1])

    return dict(srcT=srcT, dstT=dstT, flags=flags, endm=endm, fbwd=fbwd), slots, eids


_CACHE = {}


def run(inputs, trace=False):
    src = np.asarray(inputs["src_feat"], np.float32)
    dstf = np.asarray(inputs["dst_feat"], np.float32)
    edge_dst = np.asarray(inputs["edge_dst"]).astype(np.int64)
    assert src.shape == (N_EDGES, D) and dstf.shape == (N_NODES, D)

    P = _pack(edge_dst)

    wsrc = np.asarray(inputs["W_src"], np.float32).astype(np.float16)
    wdst = np.asarray(inputs["W_dst"], np.float32).astype(np.float16)
    w1 = np.asarray(inputs["W1"], np.float32).astype(np.float16)
    w2v = np.asarray(inputs["W2"], np.float32).astype(np.float16).reshape(D)
    w2pad = np.zeros((D, LGB * 32), np.float16)
    for k in range(LGB):
        w2pad[:, 32 * k + k] = w2v
    bsum = (np.asarray(inputs["b_src"], np.float32)
            + np.asarray(inputs["b_dst"], np.float32)).reshape(D, 1)
    b1 = np.asarray(inputs["b1"], np.float32).reshape(D, 1)
    bexp = np.full((D, 1), float(np.asarray(inputs["b2"]).reshape(-1)[0]) - SHIFT,
                   np.float32)

    in_maps = []
    recov = []
    for d in range(CORES):
        dv, slots, eids = _device_inputs(P, src, dstf, edge_dst, d)
        dv.update(wsrc=wsrc, wdst=wdst, w1=w1, w2pad=w2pad, bsum=bsum, b1=b1,
                  bexp=bexp)
        in_maps.append(dv)
        recov.append((slots, eids))

    if "nc" not in _CACHE:
        _CACHE["nc"] = build_nc()
    nc = _CACHE["nc"]

    res = run_bass_kernel_spmd(nc, in_maps, list(range(CORES)), trace=trace)

    out = np.empty(N_EDGES, np.float32)
    for d in range(CORES):
        slots, eids = recov[d]
        vals = np.asarray(res.results[d]["out"], np.float32).reshape(-1)
        out[eids] = vals[slots]
    _CACHE["exec_time_ns"] = res.exec_time_ns
    _CACHE["trace_path"] = (res.instructions_and_trace or (None, None))[1]
    return out[:, None]


def kernel(**inputs):
    return run(inputs, trace=bool(os.environ.get("BASS_TRACE")))


# revision 24
# speedup vs baseline: 1.4859x; 1.0100x over previous
"""Trainium2 Bass kernel for nn_AttentionWithEpinions (GNN edge attention with
segment softmax over destination nodes), 8 NeuronCores.

Strategy (graph partitioning by destination node, per the sharding hint):
- Host sorts edges by destination and bin-packs whole destination segments
  into 1024 partition-rows (8 devices x 128 rows x F slots). Every segment is
  fully contained in one partition-row of one device, so the segment softmax
  is entirely local: no collectives. The per-edge operand streams (src
  features and the destination feature of each edge) are laid out by the host
  in a feature-on-partitions [d, e] fp16 layout.
- Per device, per 1024-slot superblock:
    score^T = W_src^T @ src^T  (+)  W_dst^T @ dstf^T       (PSUM accumulate)
    a1 = Lrelu(score + b_src + b_dst)                      (evict to fp16)
    h  = W1^T @ a1                                         (PSUM)
    a2 = Lrelu(h + b1)                                     (fp16)
    logits-accumulation: col-tiled M=32 matmuls with a one-hot-padded w2;
      17 superblocks' logits land in distinct rows of one PSUM bank
      (other rows accumulate exact zeros), evicted once per block.
- Segment softmax via segmented scans on the [128, F] slot grid:
  forward scan accumulates per-segment sums, a reversed scan broadcasts each
  segment's total back to its slots, then attn = ex / total. exp() uses a
  constant shift instead of the per-segment max (cancels exactly in the
  softmax ratio; logits are O(+-5) so fp32 exp is safe).
"""

import os
import numpy as np

import concourse.bass as bass
import concourse.mybir as mybir
import concourse.tile as tile
from concourse import bacc
from concourse.bass_utils import run_bass_kernel_spmd

# ---------------- compile-time configuration ----------------
D = 128
CORES = 8
F = 1632                  # slots per partition row
EPAD = 128 * F            # 208896 slots per device
SB = 1024                 # superblock (slots) flowing through PSUM together
NSB = EPAD // SB          # 204
TILE = 512                # matmul moving free dim
LGB = 17                  # superblocks whose logits accumulate into one PSUM bank
NLGB = NSB // LGB         # 12 logit blocks
SHIFT = 16.0              # exp() stability shift (cancels in the softmax)
N_NODES = 50000
N_EDGES = 1600000

f32 = mybir.dt.float32
f16 = mybir.dt.float16

Lrelu = mybir.ActivationFunctionType.Lrelu
Exp = mybir.ActivationFunctionType.Exp
Copy = mybir.ActivationFunctionType.Copy
ADD = mybir.AluOpType.add
MULT = mybir.AluOpType.mult
MAX = mybir.AluOpType.max


def _a2_on_act(sb):
    """a1 evictions always run on ACT (they gate mm2, the loop-critical
    path). a2 evictions (which only gate the laggable logits accumulation)
    go mostly to DVE; ACT absorbs ~3/8 of them for balance."""
    return sb % 8 in (0, 3, 6)


def build_nc():
    nc = bacc.Bacc("TRN2", target_bir_lowering=False, debug=False)

    srcT_d = nc.dram_tensor("srcT", [128, EPAD], f16, kind="ExternalInput")
    dstT_d = nc.dram_tensor("dstT", [128, EPAD], f16, kind="ExternalInput")
    flags_d = nc.dram_tensor("flags", [128, F], f32, kind="ExternalInput")
    endm_d = nc.dram_tensor("endm", [128, F], f32, kind="ExternalInput")
    fbwd_d = nc.dram_tensor("fbwd", [128, F], f32, kind="ExternalInput")
    wsrc_d = nc.dram_tensor("wsrc", [D, D], f16, kind="ExternalInput")
    wdst_d = nc.dram_tensor("wdst", [D, D], f16, kind="ExternalInput")
    w1_d = nc.dram_tensor("w1", [D, D], f16, kind="ExternalInput")
    w2pad_d = nc.dram_tensor("w2pad", [D, LGB * 32], f16, kind="ExternalInput")
    bsum_d = nc.dram_tensor("bsum", [D, 1], f32, kind="ExternalInput")
    b1_d = nc.dram_tensor("b1", [D, 1], f32, kind="ExternalInput")
    bexp_d = nc.dram_tensor("bexp", [D, 1], f32, kind="ExternalInput")

    out_d = nc.dram_tensor("out", [128, F], f32, kind="ExternalOutput")
    lg_d = nc.dram_tensor("lg_scratch", [EPAD], f32)  # internal DRAM staging

    TPS = SB // TILE  # tiles per superblock (2)

    with tile.TileContext(nc) as tc:
        with tc.tile_pool(name="const", bufs=1) as cst:
            wsrc_s = cst.tile([D, D], f16)
            wdst_s = cst.tile([D, D], f16)
            w1_s = cst.tile([D, D], f16)
            w2pad_s = cst.tile([D, LGB * 32], f16)
            bsum_s = cst.tile([D, 1], f32)
            b1_s = cst.tile([D, 1], f32)
            bexp_s = cst.tile([D, 1], f32)
            for s, d in [(wsrc_s, wsrc_d), (wdst_s, wdst_d), (w1_s, w1_d),
                         (w2pad_s, w2pad_d), (bsum_s, bsum_d), (b1_s, b1_d),
                         (bexp_s, bexp_d)]:
                nc.sync.dma_start(s[:], d[:])

            # ---------------- phase 1: per-edge MLP -> logits ----------------
            with tc.tile_pool(name="stream", bufs=3) as stp, \
                 tc.tile_pool(name="act", bufs=4) as actp, \
                 tc.tile_pool(name="lgst", bufs=2) as lgstp, \
                 tc.tile_pool(name="ps", bufs=3, space="PSUM") as psp, \
                 tc.tile_pool(name="pslg", bufs=2, space="PSUM") as pslgp:
                lgp = None
                st4 = dt4 = None
                PAIR_BLK = 2 * LGB  # 34 superblocks accumulate per logits bank
                for pi in range(NSB // 2):
                    sb0 = 2 * pi
                    if sb0 % 4 == 0:
                        o4 = sb0 * SB
                        st4 = stp.tile([128, 4 * SB], f16, tag="st4")
                        nc.sync.dma_start(st4[:], srcT_d[:, o4 : o4 + 4 * SB])
                        dt4 = stp.tile([128, 4 * SB], f16, tag="dt4")
                        nc.sync.dma_start(dt4[:], dstT_d[:, o4 : o4 + 4 * SB])
                    q = (sb0 % 4) * SB
                    sts = [st4[:, q : q + SB], st4[:, q + SB : q + 2 * SB]]
                    dts = [dt4[:, q : q + SB], dt4[:, q + SB : q + 2 * SB]]

                    scores = [psp.tile([128, SB], f32, tag="ps", name=f"score{pi}_{i}") for i in range(2)]
                    # weight-phase grouped matmuls: all W_src, then all W_dst
                    for p in range(2):
                        for t in range(TPS):
                            nc.tensor.matmul(
                                scores[p][:, t * TILE : (t + 1) * TILE],
                                wsrc_s[:], sts[p][:, t * TILE : (t + 1) * TILE],
                                start=True, stop=False)
                    for p in range(2):
                        for t in range(TPS):
                            nc.tensor.matmul(
                                scores[p][:, t * TILE : (t + 1) * TILE],
                                wdst_s[:], dts[p][:, t * TILE : (t + 1) * TILE],
                                start=False, stop=True)

                    a1s = [actp.tile([128, SB], f16, tag="a1", name=f"a1_{pi}_{i}") for i in range(2)]
                    for p in range(2):
                        for t in range(TPS):
                            nc.scalar.activation(
                                a1s[p][:, t * TILE : (t + 1) * TILE],
                                scores[p][:, t * TILE : (t + 1) * TILE],
                                Lrelu, bias=bsum_s[:], scale=1.0, alpha=0.01)

                    hs = []
                    for p in range(2):
                        h = psp.tile([128, SB], f32, tag="ps", name=f"h{pi}_{p}")
                        hs.append(h)
                        for t in range(TPS):
                            nc.tensor.matmul(
                                h[:, t * TILE : (t + 1) * TILE],
                                w1_s[:], a1s[p][:, t * TILE : (t + 1) * TILE],
                                start=True, stop=True)

                    a2s = [actp.tile([128, SB], f16, tag="a2", name=f"a2_{pi}_{i}") for i in range(2)]
                    for p in range(2):
                        sb = sb0 + p
                        if sb % 2 == 0:
                            nc.scalar.activation(a2s[p][:], hs[p][:], Lrelu,
                                                 bias=b1_s[:], scale=1.0, alpha=0.01)
                        else:
                            a2t = actp.tile([128, SB], f16, tag="a2t", name=f"a2t_{pi}_{p}")
                            nc.vector.tensor_scalar(a2t[:], hs[p][:], b1_s[:], None, ADD)
                            nc.vector.scalar_tensor_tensor(a2s[p][:], a2t[:], 0.01, a2t[:], MULT, MAX)

                    # logits: 4-way col-tiled accumulating matmuls (M=32).
                    # Column group j = 2*t + p holds pair-member p, tile t;
                    # within a block, pair k of LGB lands on row 32*j + k.
                    k = pi % LGB
                    if k == 0:
                        lgp = pslgp.tile([128, TILE], f32, tag="lg")
                    for p in range(2):
                        for t in range(TPS):
                            j = 2 * t + p
                            nc.tensor.matmul(
                                lgp[32 * j : 32 * j + 32, :],
                                w2pad_s[:, 32 * k : 32 * (k + 1)],
                                a2s[p][:, t * TILE : (t + 1) * TILE],
                                start=(k == 0), stop=(k == LGB - 1),
                                tile_position=(0, 32 * j))
                    if k == LGB - 1:
                        blk = pi // LGB
                        lgs = lgstp.tile([128, TILE], f32, tag="lgs")
                        nc.scalar.activation(lgs[:], lgp[:], Copy)
                        # row 32*(2t+p)+k  ->  sb = blk*PAIR_BLK + 2k + p, tile t
                        lgv = lg_d[:].rearrange("(j t f) -> j t f", t=TPS, f=TILE)
                        for p in range(2):
                            for t in range(TPS):
                                j = 2 * t + p
                                nc.sync.dma_start(
                                    lgv[blk * PAIR_BLK + p : blk * PAIR_BLK + p + 2 * LGB - 1 : 2, t, :],
                                    lgs[32 * j : 32 * j + LGB, :])

            # ---------------- phase 2: segment softmax ----------------
            with tc.tile_pool(name="soft", bufs=1) as sfp:
                lgsc = sfp.tile([128, F], f32)
                nc.sync.dma_start(lgsc[:], lg_d[:].rearrange("(p f) -> p f", p=128))
                flags_s = sfp.tile([128, F], f32)
                endm_s = sfp.tile([128, F], f32)
                fbwd_s = sfp.tile([128, F], f32)
                nc.sync.dma_start(flags_s[:], flags_d[:])
                nc.sync.dma_start(endm_s[:], endm_d[:])
                nc.sync.dma_start(fbwd_s[:], fbwd_d[:])

                ex = sfp.tile([128, F], f32)
                nc.scalar.activation(ex[:], lgsc[:], Exp, bias=bexp_s[:], scale=1.0)

                S = sfp.tile([128, F], f32)
                nc.vector.tensor_tensor_scan(S[:], flags_s[:], ex[:], 0.0, MULT, ADD)
                dend = sfp.tile([128, F], f32)
                nc.vector.tensor_tensor(dend[:], S[:], endm_s[:], MULT)
                Trev = sfp.tile([128, F], f32)
                nc.vector.tensor_tensor_scan(Trev[:], fbwd_s[:], dend[:, ::-1], 0.0, MULT, ADD)
                R = sfp.tile([128, F], f32)
                nc.vector.reciprocal(R[:], Trev[:])
                attn = sfp.tile([128, F], f32)
                nc.vector.tensor_tensor(attn[:], ex[:], R[:, ::-1], MULT)
                nc.sync.dma_start(out_d[:], attn[:])

    nc.finalize()
    return nc


# ---------------- host-side packing ----------------

def _pack(edge_dst):
    order = np.argsort(edge_dst, kind="stable")
    sdst = edge_dst[order].astype(np.int64)
    counts = np.bincount(edge_dst, minlength=N_NODES).astype(np.int64)

    row_of_node = np.empty(N_NODES, np.int64)
    col_of_node = np.empty(N_NODES, np.int64)
    row, col = 0, 0
    for n in range(N_NODES):
        c = counts[n]
        if col + c > F:
            row += 1
            col = 0
        row_of_node[n] = row
        col_of_node[n] = col
        col += c
    assert row < 128 * CORES, f"packing overflow: {row}"

    starts = np.cumsum(counts) - counts
    within = np.arange(N_EDGES, dtype=np.int64) - starts[sdst]
    slot_global = row_of_node[sdst] * F + col_of_node[sdst] + within
    dev_of_edge = (row_of_node[sdst] // 128).astype(np.int64)
    slot_in_dev = slot_global - dev_of_edge * EPAD
    return dict(order=order, sdst=sdst, dev_of_edge=dev_of_edge,
                slot_in_dev=slot_in_dev)


def _device_inputs(P, src, dstf, edge_dst, d):
    mask = P["dev_of_edge"] == d
    slots = P["slot_in_dev"][mask]
    eids = P["order"][mask]

    srcT = np.zeros((EPAD, D), np.float16)
    srcT[slots] = src[eids].astype(np.float16)
    srcT = np.ascontiguousarray(srcT.T)

    dstT = np.zeros((EPAD, D), np.float16)
    dstT[slots] = dstf[edge_dst[eids]].astype(np.float16)
    dstT = np.ascontiguousarray(dstT.T)

    used = np.zeros(EPAD, bool)
    used[slots] = True
    fl = np.ones(EPAD, np.float32)
    sd = P["sdst"][mask]
    seg_start_slots = slots[np.concatenate([[True], sd[1:] != sd[:-1]])]
    fl[seg_start_slots] = 0.0
    prev_used = np.concatenate([[False], used[:-1]])
    run_start = (~used) & (prev_used | (np.arange(EPAD) % F == 0))
    fl[run_start] = 0.0
    fl[np.arange(0, EPAD, F)] = 0.0
    flags = fl.reshape(128, F)

    nxt_reset = np.concatenate([flags[:, 1:], np.zeros((128, 1), np.float32)], axis=1)
    endm = np.where(nxt_reset == 0.0, 1.0, 0.0).astype(np.float32)
    fbwd = np.ascontiguousarray((1.0 - endm)[:, ::-1])

    return dict(srcT=srcT, dstT=dstT, flags=flags, endm=endm, fbwd=fbwd), slots, eids


_CACHE = {}


def run(inputs, trace=False):
    src = np.asarray(inputs["src_feat"], np.float32)
    dstf = np.asarray(inputs["dst_feat"], np.float32)
    edge_dst = np.asarray(inputs["edge_dst"]).astype(np.int64)
    assert src.shape == (N_EDGES, D) and dstf.shape == (N_NODES, D)

    P = _pack(edge_dst)

    wsrc = np.asarray(inputs["W_src"], np.float32).astype(np.float16)
    wdst = np.asarray(inputs["W_dst"], np.float32).astype(np.float16)
    w1 = np.asarray(inputs["W1"], np.float32).astype(np.float16)
    w2v = np.asarray(inputs["W2"], np.float32).astype(np.float16).reshape(D)
    w2pad = np.zeros((D, LGB * 32), np.float16)
    for k in range(LGB):
        w2pad[:, 32 * k + k] = w2v
    bsum = (np.asarray(inputs["b_src"], np.float32)
            + np.asarray(inputs["b_dst"], np.float32)).reshape(D, 1)
    b1 = np.asarray(inputs["b1"], np.float32).reshape(D, 1)
    bexp = np.full((D, 1), float(np.asarray(inputs["b2"]).reshape(-1)[0]) - SHIFT,
                   np.float32)

    in_maps = []
    recov = []
    for d in range(CORES):
        dv, slots, eids = _device_inputs(P, src, dstf, edge_dst, d)
        dv.update(wsrc=wsrc, wdst=wdst, w1=w1, w2pad=w2pad, bsum=bsum, b1=b1,
                  bexp=bexp)
        in_maps.append(dv)
        recov.append((slots, eids))

    if "nc" not in _CACHE:
        _CACHE["nc"] = build_nc()
    nc = _CACHE["nc"]

    res = run_bass_kernel_spmd(nc, in_maps, list(range(CORES)), trace=trace)

    out = np.empty(N_EDGES, np.float32)
    for d in range(CORES):
        slots, eids = recov[d]
        vals = np.asarray(res.results[d]["out"], np.float32).reshape(-1)
        out[eids] = vals[slots]
    _CACHE["exec_time_ns"] = res.exec_time_ns
    _CACHE["trace_path"] = (res.instructions_and_trace or (None, None))[1]
    return out[:, None]


def kernel(**inputs):
    return run(inputs, trace=bool(os.environ.get("BASS_TRACE")))


# revision 25
# speedup vs baseline: 1.5081x; 1.0149x over previous
"""Trainium2 Bass kernel for nn_AttentionWithEpinions (GNN edge attention with
segment softmax over destination nodes), 8 NeuronCores.

Strategy (graph partitioning by destination node, per the sharding hint):
- Host sorts edges by destination and bin-packs whole destination segments
  into 1024 partition-rows (8 devices x 128 rows x F slots). Every segment is
  fully contained in one partition-row of one device, so the segment softmax
  is entirely local: no collectives. The per-edge operand streams (src
  features and the destination feature of each edge) are laid out by the host
  in a feature-on-partitions [d, e] fp16 layout.
- Per device, per 1024-slot superblock:
    score^T = W_src^T @ src^T  (+)  W_dst^T @ dstf^T       (PSUM accumulate)
    a1 = Lrelu(score + b_src + b_dst)                      (evict to fp16)
    h  = W1^T @ a1                                         (PSUM)
    a2 = Lrelu(h + b1)                                     (fp16)
    logits-accumulation: col-tiled M=32 matmuls with a one-hot-padded w2;
      17 superblocks' logits land in distinct rows of one PSUM bank
      (other rows accumulate exact zeros), evicted once per block.
- Segment softmax via segmented scans on the [128, F] slot grid:
  forward scan accumulates per-segment sums, a reversed scan broadcasts each
  segment's total back to its slots, then attn = ex / total. exp() uses a
  constant shift instead of the per-segment max (cancels exactly in the
  softmax ratio; logits are O(+-5) so fp32 exp is safe).
"""

import os
import numpy as np

import concourse.bass as bass
import concourse.mybir as mybir
import concourse.tile as tile
from concourse import bacc
from concourse.bass_utils import run_bass_kernel_spmd

# ---------------- compile-time configuration ----------------
D = 128
CORES = 8
F = 1600                  # slots per partition row
EPAD = 128 * F            # 208896 slots per device
SB = 1024                 # superblock (slots) flowing through PSUM together
NSB = EPAD // SB          # 200
TILE = 512                # matmul moving free dim
LGB = 25                  # superblock-pairs whose logits accumulate into one PSUM bank
NLGB = NSB // 2 // LGB    # 4 logit blocks
SHIFT = 16.0              # exp() stability shift (cancels in the softmax)
N_NODES = 50000
N_EDGES = 1600000

f32 = mybir.dt.float32
f16 = mybir.dt.float16

Lrelu = mybir.ActivationFunctionType.Lrelu
Exp = mybir.ActivationFunctionType.Exp
Copy = mybir.ActivationFunctionType.Copy
ADD = mybir.AluOpType.add
MULT = mybir.AluOpType.mult
MAX = mybir.AluOpType.max


def _a2_on_act(sb):
    """a1 evictions always run on ACT (they gate mm2, the loop-critical
    path). a2 evictions (which only gate the laggable logits accumulation)
    go mostly to DVE; ACT absorbs ~3/8 of them for balance."""
    return sb % 8 in (0, 3, 6)


def build_nc():
    nc = bacc.Bacc("TRN2", target_bir_lowering=False, debug=False)

    srcT_d = nc.dram_tensor("srcT", [128, EPAD], f16, kind="ExternalInput")
    dstT_d = nc.dram_tensor("dstT", [128, EPAD], f16, kind="ExternalInput")
    flags_d = nc.dram_tensor("flags", [128, F], f32, kind="ExternalInput")
    endm_d = nc.dram_tensor("endm", [128, F], f32, kind="ExternalInput")
    fbwd_d = nc.dram_tensor("fbwd", [128, F], f32, kind="ExternalInput")
    wsrc_d = nc.dram_tensor("wsrc", [D, D], f16, kind="ExternalInput")
    wdst_d = nc.dram_tensor("wdst", [D, D], f16, kind="ExternalInput")
    w1_d = nc.dram_tensor("w1", [D, D], f16, kind="ExternalInput")
    w2pad_d = nc.dram_tensor("w2pad", [D, LGB * 32], f16, kind="ExternalInput")
    bsum_d = nc.dram_tensor("bsum", [D, 1], f32, kind="ExternalInput")
    b1_d = nc.dram_tensor("b1", [D, 1], f32, kind="ExternalInput")
    bexp_d = nc.dram_tensor("bexp", [D, 1], f32, kind="ExternalInput")

    out_d = nc.dram_tensor("out", [128, F], f32, kind="ExternalOutput")
    lg_d = nc.dram_tensor("lg_scratch", [EPAD], f32)  # internal DRAM staging

    TPS = SB // TILE  # tiles per superblock (2)

    with tile.TileContext(nc) as tc:
        with tc.tile_pool(name="const", bufs=1) as cst:
            wsrc_s = cst.tile([D, D], f16)
            wdst_s = cst.tile([D, D], f16)
            w1_s = cst.tile([D, D], f16)
            w2pad_s = cst.tile([D, LGB * 32], f16)
            bsum_s = cst.tile([D, 1], f32)
            b1_s = cst.tile([D, 1], f32)
            bexp_s = cst.tile([D, 1], f32)
            flags_s = cst.tile([128, F], f32)
            endm_s = cst.tile([128, F], f32)
            fbwd_s = cst.tile([128, F], f32)
            for s, d in [(wsrc_s, wsrc_d), (wdst_s, wdst_d), (w1_s, w1_d),
                         (w2pad_s, w2pad_d), (bsum_s, bsum_d), (b1_s, b1_d),
                         (bexp_s, bexp_d), (flags_s, flags_d), (endm_s, endm_d),
                         (fbwd_s, fbwd_d)]:
                nc.sync.dma_start(s[:], d[:])

            # ---------------- phase 1: per-edge MLP -> logits ----------------
            with tc.tile_pool(name="stream", bufs=3) as stp, \
                 tc.tile_pool(name="act", bufs=4) as actp, \
                 tc.tile_pool(name="lgst", bufs=2) as lgstp, \
                 tc.tile_pool(name="ps", bufs=3, space="PSUM") as psp, \
                 tc.tile_pool(name="pslg", bufs=2, space="PSUM") as pslgp:
                lgp = None
                st4 = dt4 = None
                PAIR_BLK = 2 * LGB  # 34 superblocks accumulate per logits bank
                for pi in range(NSB // 2):
                    sb0 = 2 * pi
                    if sb0 % 4 == 0:
                        o4 = sb0 * SB
                        st4 = stp.tile([128, 4 * SB], f16, tag="st4")
                        nc.sync.dma_start(st4[:], srcT_d[:, o4 : o4 + 4 * SB])
                        dt4 = stp.tile([128, 4 * SB], f16, tag="dt4")
                        nc.sync.dma_start(dt4[:], dstT_d[:, o4 : o4 + 4 * SB])
                    q = (sb0 % 4) * SB
                    sts = [st4[:, q : q + SB], st4[:, q + SB : q + 2 * SB]]
                    dts = [dt4[:, q : q + SB], dt4[:, q + SB : q + 2 * SB]]

                    scores = [psp.tile([128, SB], f32, tag="ps", name=f"score{pi}_{i}") for i in range(2)]
                    # weight-phase grouped matmuls: all W_src, then all W_dst
                    for p in range(2):
                        for t in range(TPS):
                            nc.tensor.matmul(
                                scores[p][:, t * TILE : (t + 1) * TILE],
                                wsrc_s[:], sts[p][:, t * TILE : (t + 1) * TILE],
                                start=True, stop=False)
                    for p in range(2):
                        for t in range(TPS):
                            nc.tensor.matmul(
                                scores[p][:, t * TILE : (t + 1) * TILE],
                                wdst_s[:], dts[p][:, t * TILE : (t + 1) * TILE],
                                start=False, stop=True)

                    a1s = [actp.tile([128, SB], f16, tag="a1", name=f"a1_{pi}_{i}") for i in range(2)]
                    for p in range(2):
                        for t in range(TPS):
                            nc.scalar.activation(
                                a1s[p][:, t * TILE : (t + 1) * TILE],
                                scores[p][:, t * TILE : (t + 1) * TILE],
                                Lrelu, bias=bsum_s[:], scale=1.0, alpha=0.01)

                    hs = []
                    for p in range(2):
                        h = psp.tile([128, SB], f32, tag="ps", name=f"h{pi}_{p}")
                        hs.append(h)
                        for t in range(TPS):
                            nc.tensor.matmul(
                                h[:, t * TILE : (t + 1) * TILE],
                                w1_s[:], a1s[p][:, t * TILE : (t + 1) * TILE],
                                start=True, stop=True)

                    a2s = [actp.tile([128, SB], f16, tag="a2", name=f"a2_{pi}_{i}") for i in range(2)]
                    for p in range(2):
                        sb = sb0 + p
                        if sb % 2 == 0:
                            nc.scalar.activation(a2s[p][:], hs[p][:], Lrelu,
                                                 bias=b1_s[:], scale=1.0, alpha=0.01)
                        else:
                            a2t = actp.tile([128, SB], f16, tag="a2t", name=f"a2t_{pi}_{p}")
                            nc.vector.tensor_scalar(a2t[:], hs[p][:], b1_s[:], None, ADD)
                            nc.vector.scalar_tensor_tensor(a2s[p][:], a2t[:], 0.01, a2t[:], MULT, MAX)

                    # logits: 4-way col-tiled accumulating matmuls (M=32).
                    # Column group j = 2*t + p holds pair-member p, tile t;
                    # within a block, pair k of LGB lands on row 32*j + k.
                    k = pi % LGB
                    if k == 0:
                        lgp = pslgp.tile([128, TILE], f32, tag="lg")
                    for p in range(2):
                        for t in range(TPS):
                            j = 2 * t + p
                            nc.tensor.matmul(
                                lgp[32 * j : 32 * j + 32, :],
                                w2pad_s[:, 32 * k : 32 * (k + 1)],
                                a2s[p][:, t * TILE : (t + 1) * TILE],
                                start=(k == 0), stop=(k == LGB - 1),
                                tile_position=(0, 32 * j))
                    if k == LGB - 1:
                        blk = pi // LGB
                        lgs = lgstp.tile([128, TILE], f32, tag="lgs")
                        nc.scalar.activation(lgs[:], lgp[:], Copy)
                        # row 32*(2t+p)+k  ->  sb = blk*PAIR_BLK + 2k + p, tile t
                        lgv = lg_d[:].rearrange("(j t f) -> j t f", t=TPS, f=TILE)
                        for p in range(2):
                            for t in range(TPS):
                                j = 2 * t + p
                                nc.sync.dma_start(
                                    lgv[blk * PAIR_BLK + p : blk * PAIR_BLK + p + 2 * LGB - 1 : 2, t, :],
                                    lgs[32 * j : 32 * j + LGB, :])

            # ---------------- phase 2: segment softmax ----------------
            with tc.tile_pool(name="soft", bufs=1) as sfp:
                lgsc = sfp.tile([128, F], f32)
                nc.sync.dma_start(lgsc[:], lg_d[:].rearrange("(p f) -> p f", p=128))

                ex = sfp.tile([128, F], f32)
                nc.scalar.activation(ex[:], lgsc[:], Exp, bias=bexp_s[:], scale=1.0)

                S = sfp.tile([128, F], f32)
                nc.vector.tensor_tensor_scan(S[:], flags_s[:], ex[:], 0.0, MULT, ADD)
                dend = sfp.tile([128, F], f32)
                nc.vector.tensor_tensor(dend[:], S[:], endm_s[:], MULT)
                Trev = sfp.tile([128, F], f32)
                nc.vector.tensor_tensor_scan(Trev[:], fbwd_s[:], dend[:, ::-1], 0.0, MULT, ADD)
                R = sfp.tile([128, F], f32)
                nc.vector.reciprocal(R[:], Trev[:])
                attn = sfp.tile([128, F], f32)
                nc.vector.tensor_tensor(attn[:], ex[:], R[:, ::-1], MULT)
                nc.sync.dma_start(out_d[:], attn[:])

    nc.finalize()
    return nc


# ---------------- host-side packing ----------------

def _pack(edge_dst):
    order = np.argsort(edge_dst, kind="stable")
    sdst = edge_dst[order].astype(np.int64)
    counts = np.bincount(edge_dst, minlength=N_NODES).astype(np.int64)

    row_of_node = np.empty(N_NODES, np.int64)
    col_of_node = np.empty(N_NODES, np.int64)
    row, col = 0, 0
    for n in range(N_NODES):
        c = counts[n]
        if col + c > F:
            row += 1
            col = 0
        row_of_node[n] = row
        col_of_node[n] = col
        col += c
    assert row < 128 * CORES, f"packing overflow: {row}"

    starts = np.cumsum(counts) - counts
    within = np.arange(N_EDGES, dtype=np.int64) - starts[sdst]
    slot_global = row_of_node[sdst] * F + col_of_node[sdst] + within
    dev_of_edge = (row_of_node[sdst] // 128).astype(np.int64)
    slot_in_dev = slot_global - dev_of_edge * EPAD
    return dict(order=order, sdst=sdst, dev_of_edge=dev_of_edge,
                slot_in_dev=slot_in_dev)


def _device_inputs(P, src, dstf, edge_dst, d):
    mask = P["dev_of_edge"] == d
    slots = P["slot_in_dev"][mask]
    eids = P["order"][mask]

    srcT = np.zeros((EPAD, D), np.float16)
    srcT[slots] = src[eids].astype(np.float16)
    srcT = np.ascontiguousarray(srcT.T)

    dstT = np.zeros((EPAD, D), np.float16)
    dstT[slots] = dstf[edge_dst[eids]].astype(np.float16)
    dstT = np.ascontiguousarray(dstT.T)

    used = np.zeros(EPAD, bool)
    used[slots] = True
    fl = np.ones(EPAD, np.float32)
    sd = P["sdst"][mask]
    seg_start_slots = slots[np.concatenate([[True], sd[1:] != sd[:-1]])]
    fl[seg_start_slots] = 0.0
    prev_used = np.concatenate([[False], used[:-1]])
    run_start = (~used) & (prev_used | (np.arange(EPAD) % F == 0))
    fl[run_start] = 0.0
    fl[np.arange(0, EPAD, F)] = 0.0
    flags = fl.reshape(128, F)

    nxt_reset = np.concatenate([flags[:, 1:], np.zeros((128, 1), np.float32)], axis=1)
    endm = np.where(nxt_reset == 0.0, 1.0, 0.0).astype(np.float32)
    fbwd = np.ascontiguousarray((1.0 - endm)[:, ::-1])

    return dict(srcT=srcT, dstT=dstT, flags=flags, endm=endm, fbwd=fbwd), slots, eids


_CACHE = {}


def run(inputs, trace=False):
    src = np.asarray(inputs["src_feat"], np.float32)
    dstf = np.asarray(inputs["dst_feat"], np.float32)
    edge_dst = np.asarray(inputs["edge_dst"]).astype(np.int64)
    assert src.shape == (N_EDGES, D) and dstf.shape == (N_NODES, D)

    P = _pack(edge_dst)

    wsrc = np.asarray(inputs["W_src"], np.float32).astype(np.float16)
    wdst = np.asarray(inputs["W_dst"], np.float32).astype(np.float16)
    w1 = np.asarray(inputs["W1"], np.float32).astype(np.float16)
    w2v = np.asarray(inputs["W2"], np.float32).astype(np.float16).reshape(D)
    w2pad = np.zeros((D, LGB * 32), np.float16)
    for k in range(LGB):
        w2pad[:, 32 * k + k] = w2v
    bsum = (np.asarray(inputs["b_src"], np.float32)
            + np.asarray(inputs["b_dst"], np.float32)).reshape(D, 1)
    b1 = np.asarray(inputs["b1"], np.float32).reshape(D, 1)
    bexp = np.full((D, 1), float(np.asarray(inputs["b2"]).reshape(-1)[0]) - SHIFT,
                   np.float32)

    in_maps = []
    recov = []
    for d in range(CORES):
        dv, slots, eids = _device_inputs(P, src, dstf, edge_dst, d)
        dv.update(wsrc=wsrc, wdst=wdst, w1=w1, w2pad=w2pad, bsum=bsum, b1=b1,
                  bexp=bexp)
        in_maps.append(dv)
        recov.append((slots, eids))

    if "nc" not in _CACHE:
        _CACHE["nc"] = build_nc()
    nc = _CACHE["nc"]

    res = run_bass_kernel_spmd(nc, in_maps, list(range(CORES)), trace=trace)

    out = np.empty(N_EDGES, np.float32)
    for d in range(CORES):
        slots, eids = recov[d]
        vals = np.asarray(res.results[d]["out"], np.float32).reshape(-1)
        out[eids] = vals[slots]
    _CACHE["exec_time_ns"] = res.exec_time_ns
    _CACHE["trace_path"] = (res.instructions_and_trace or (None, None))[1]
    return out[:, None]


def kernel(**inputs):
    return run(inputs, trace=bool(os.environ.get("BASS_TRACE")))


# revision 26
# speedup vs baseline: 1.5082x; 1.0001x over previous
"""Trainium2 Bass kernel for nn_AttentionWithEpinions (GNN edge attention with
segment softmax over destination nodes), 8 NeuronCores.

Strategy (graph partitioning by destination node, per the sharding hint):
- Host sorts edges by destination and bin-packs whole destination segments
  into 1024 partition-rows (8 devices x 128 rows x F slots). Every segment is
  fully contained in one partition-row of one device, so the segment softmax
  is entirely local: no collectives. The per-edge operand streams (src
  features and the destination feature of each edge) are laid out by the host
  in a feature-on-partitions [d, e] fp16 layout.
- Per device, per 1024-slot superblock:
    score^T = W_src^T @ src^T  (+)  W_dst^T @ dstf^T       (PSUM accumulate)
    a1 = Lrelu(score + b_src + b_dst)                      (evict to fp16)
    h  = W1^T @ a1                                         (PSUM)
    a2 = Lrelu(h + b1)                                     (fp16)
    logits-accumulation: col-tiled M=32 matmuls with a one-hot-padded w2;
      17 superblocks' logits land in distinct rows of one PSUM bank
      (other rows accumulate exact zeros), evicted once per block.
- Segment softmax via segmented scans on the [128, F] slot grid:
  forward scan accumulates per-segment sums, a reversed scan broadcasts each
  segment's total back to its slots, then attn = ex / total. exp() uses a
  constant shift instead of the per-segment max (cancels exactly in the
  softmax ratio; logits are O(+-5) so fp32 exp is safe).
"""

import os
import numpy as np

import concourse.bass as bass
import concourse.mybir as mybir
import concourse.tile as tile
from concourse import bacc
from concourse.bass_utils import run_bass_kernel_spmd

# ---------------- compile-time configuration ----------------
D = 128
CORES = 8
F = 1600                  # slots per partition row
EPAD = 128 * F            # 208896 slots per device
SB = 1024                 # superblock (slots) flowing through PSUM together
NSB = EPAD // SB          # 200
TILE = 512                # matmul moving free dim
LGB = 25                  # superblock-pairs whose logits accumulate into one PSUM bank
NLGB = NSB // 2 // LGB    # 4 logit blocks
SHIFT = 16.0              # exp() stability shift (cancels in the softmax)
N_NODES = 50000
N_EDGES = 1600000

f32 = mybir.dt.float32
f16 = mybir.dt.float16

Lrelu = mybir.ActivationFunctionType.Lrelu
Exp = mybir.ActivationFunctionType.Exp
Copy = mybir.ActivationFunctionType.Copy
ADD = mybir.AluOpType.add
MULT = mybir.AluOpType.mult
MAX = mybir.AluOpType.max


def _a2_on_act(sb):
    """a1 evictions always run on ACT (they gate mm2, the loop-critical
    path). a2 evictions (which only gate the laggable logits accumulation)
    go mostly to DVE; ACT absorbs ~3/8 of them for balance."""
    return sb % 8 in (0, 3, 6)


def build_nc():
    nc = bacc.Bacc("TRN2", target_bir_lowering=False, debug=False)

    srcT_d = nc.dram_tensor("srcT", [128, EPAD], f16, kind="ExternalInput")
    dstT_d = nc.dram_tensor("dstT", [128, EPAD], f16, kind="ExternalInput")
    flags_d = nc.dram_tensor("flags", [128, F], f32, kind="ExternalInput")
    endm_d = nc.dram_tensor("endm", [128, F], f32, kind="ExternalInput")
    fbwd_d = nc.dram_tensor("fbwd", [128, F], f32, kind="ExternalInput")
    wsrc_d = nc.dram_tensor("wsrc", [D, D], f16, kind="ExternalInput")
    wdst_d = nc.dram_tensor("wdst", [D, D], f16, kind="ExternalInput")
    w1_d = nc.dram_tensor("w1", [D, D], f16, kind="ExternalInput")
    w2pad_d = nc.dram_tensor("w2pad", [D, LGB * 32], f16, kind="ExternalInput")
    bsum_d = nc.dram_tensor("bsum", [D, 1], f32, kind="ExternalInput")
    b1_d = nc.dram_tensor("b1", [D, 1], f32, kind="ExternalInput")
    bexp_d = nc.dram_tensor("bexp", [D, 1], f32, kind="ExternalInput")

    out_d = nc.dram_tensor("out", [128, F], f32, kind="ExternalOutput")
    lg_d = nc.dram_tensor("lg_scratch", [EPAD], f32)  # internal DRAM staging

    TPS = SB // TILE  # tiles per superblock (2)

    with tile.TileContext(nc) as tc:
        with tc.tile_pool(name="const", bufs=1) as cst:
            wsrc_s = cst.tile([D, D], f16)
            wdst_s = cst.tile([D, D], f16)
            w1_s = cst.tile([D, D], f16)
            w2pad_s = cst.tile([D, LGB * 32], f16)
            bsum_s = cst.tile([D, 1], f32)
            b1_s = cst.tile([D, 1], f32)
            bexp_s = cst.tile([D, 1], f32)
            flags_s = cst.tile([128, F], f32)
            endm_s = cst.tile([128, F], f32)
            fbwd_s = cst.tile([128, F], f32)
            for s, d in [(wsrc_s, wsrc_d), (wdst_s, wdst_d), (w1_s, w1_d),
                         (w2pad_s, w2pad_d), (bsum_s, bsum_d), (b1_s, b1_d),
                         (bexp_s, bexp_d), (flags_s, flags_d), (endm_s, endm_d),
                         (fbwd_s, fbwd_d)]:
                nc.sync.dma_start(s[:], d[:])

            # ---------------- phase 1: per-edge MLP -> logits ----------------
            with tc.tile_pool(name="stream", bufs=4) as stp, \
                 tc.tile_pool(name="act", bufs=6) as actp, \
                 tc.tile_pool(name="lgst", bufs=2) as lgstp, \
                 tc.tile_pool(name="ps", bufs=3, space="PSUM") as psp, \
                 tc.tile_pool(name="pslg", bufs=2, space="PSUM") as pslgp:
                lgp = None
                st4 = dt4 = None
                PAIR_BLK = 2 * LGB  # 34 superblocks accumulate per logits bank
                for pi in range(NSB // 2):
                    sb0 = 2 * pi
                    if sb0 % 4 == 0:
                        o4 = sb0 * SB
                        st4 = stp.tile([128, 4 * SB], f16, tag="st4")
                        nc.sync.dma_start(st4[:], srcT_d[:, o4 : o4 + 4 * SB])
                        dt4 = stp.tile([128, 4 * SB], f16, tag="dt4")
                        nc.sync.dma_start(dt4[:], dstT_d[:, o4 : o4 + 4 * SB])
                    q = (sb0 % 4) * SB
                    sts = [st4[:, q : q + SB], st4[:, q + SB : q + 2 * SB]]
                    dts = [dt4[:, q : q + SB], dt4[:, q + SB : q + 2 * SB]]

                    scores = [psp.tile([128, SB], f32, tag="ps", name=f"score{pi}_{i}") for i in range(2)]
                    # weight-phase grouped matmuls: all W_src, then all W_dst
                    for p in range(2):
                        for t in range(TPS):
                            nc.tensor.matmul(
                                scores[p][:, t * TILE : (t + 1) * TILE],
                                wsrc_s[:], sts[p][:, t * TILE : (t + 1) * TILE],
                                start=True, stop=False)
                    for p in range(2):
                        for t in range(TPS):
                            nc.tensor.matmul(
                                scores[p][:, t * TILE : (t + 1) * TILE],
                                wdst_s[:], dts[p][:, t * TILE : (t + 1) * TILE],
                                start=False, stop=True)

                    a1s = [actp.tile([128, SB], f16, tag="a1", name=f"a1_{pi}_{i}") for i in range(2)]
                    for p in range(2):
                        for t in range(TPS):
                            nc.scalar.activation(
                                a1s[p][:, t * TILE : (t + 1) * TILE],
                                scores[p][:, t * TILE : (t + 1) * TILE],
                                Lrelu, bias=bsum_s[:], scale=1.0, alpha=0.01)

                    hs = []
                    for p in range(2):
                        h = psp.tile([128, SB], f32, tag="ps", name=f"h{pi}_{p}")
                        hs.append(h)
                        for t in range(TPS):
                            nc.tensor.matmul(
                                h[:, t * TILE : (t + 1) * TILE],
                                w1_s[:], a1s[p][:, t * TILE : (t + 1) * TILE],
                                start=True, stop=True)

                    a2s = [actp.tile([128, SB], f16, tag="a2", name=f"a2_{pi}_{i}") for i in range(2)]
                    for p in range(2):
                        sb = sb0 + p
                        if sb % 2 == 0:
                            nc.scalar.activation(a2s[p][:], hs[p][:], Lrelu,
                                                 bias=b1_s[:], scale=1.0, alpha=0.01)
                        else:
                            a2t = actp.tile([128, SB], f16, tag="a2t", name=f"a2t_{pi}_{p}")
                            nc.vector.tensor_scalar(a2t[:], hs[p][:], b1_s[:], None, ADD)
                            nc.vector.scalar_tensor_tensor(a2s[p][:], a2t[:], 0.01, a2t[:], MULT, MAX)

                    # logits: 4-way col-tiled accumulating matmuls (M=32).
                    # Column group j = 2*t + p holds pair-member p, tile t;
                    # within a block, pair k of LGB lands on row 32*j + k.
                    k = pi % LGB
                    if k == 0:
                        lgp = pslgp.tile([128, TILE], f32, tag="lg")
                    for p in range(2):
                        for t in range(TPS):
                            j = 2 * t + p
                            nc.tensor.matmul(
                                lgp[32 * j : 32 * j + 32, :],
                                w2pad_s[:, 32 * k : 32 * (k + 1)],
                                a2s[p][:, t * TILE : (t + 1) * TILE],
                                start=(k == 0), stop=(k == LGB - 1),
                                tile_position=(0, 32 * j))
                    if k == LGB - 1:
                        blk = pi // LGB
                        lgs = lgstp.tile([128, TILE], f32, tag="lgs")
                        nc.scalar.activation(lgs[:], lgp[:], Copy)
                        # row 32*(2t+p)+k  ->  sb = blk*PAIR_BLK + 2k + p, tile t
                        lgv = lg_d[:].rearrange("(j t f) -> j t f", t=TPS, f=TILE)
                        for p in range(2):
                            for t in range(TPS):
                                j = 2 * t + p
                                nc.sync.dma_start(
                                    lgv[blk * PAIR_BLK + p : blk * PAIR_BLK + p + 2 * LGB - 1 : 2, t, :],
                                    lgs[32 * j : 32 * j + LGB, :])

            # ---------------- phase 2: segment softmax ----------------
            with tc.tile_pool(name="soft", bufs=1) as sfp:
                lgsc = sfp.tile([128, F], f32)
                nc.sync.dma_start(lgsc[:], lg_d[:].rearrange("(p f) -> p f", p=128))

                ex = sfp.tile([128, F], f32)
                nc.scalar.activation(ex[:], lgsc[:], Exp, bias=bexp_s[:], scale=1.0)

                S = sfp.tile([128, F], f32)
                nc.vector.tensor_tensor_scan(S[:], flags_s[:], ex[:], 0.0, MULT, ADD)
                dend = sfp.tile([128, F], f32)
                nc.vector.tensor_tensor(dend[:], S[:], endm_s[:], MULT)
                Trev = sfp.tile([128, F], f32)
                nc.vector.tensor_tensor_scan(Trev[:], fbwd_s[:], dend[:, ::-1], 0.0, MULT, ADD)
                R = sfp.tile([128, F], f32)
                nc.vector.reciprocal(R[:], Trev[:])
                attn = sfp.tile([128, F], f32)
                nc.vector.tensor_tensor(attn[:], ex[:], R[:, ::-1], MULT)
                nc.sync.dma_start(out_d[:], attn[:])

    nc.finalize()
    return nc


# ---------------- host-side packing ----------------

def _pack(edge_dst):
    order = np.argsort(edge_dst, kind="stable")
    sdst = edge_dst[order].astype(np.int64)
    counts = np.bincount(edge_dst, minlength=N_NODES).astype(np.int64)

    row_of_node = np.empty(N_NODES, np.int64)
    col_of_node = np.empty(N_NODES, np.int64)
    row, col = 0, 0
    for n in range(N_NODES):
        c = counts[n]
        if col + c > F:
            row += 1
            col = 0
        row_of_node[n] = row
        col_of_node[n] = col
        col += c
    assert row < 128 * CORES, f"packing overflow: {row}"

    starts = np.cumsum(counts) - counts
    within = np.arange(N_EDGES, dtype=np.int64) - starts[sdst]
    slot_global = row_of_node[sdst] * F + col_of_node[sdst] + within
    dev_of_edge = (row_of_node[sdst] // 128).astype(np.int64)
    slot_in_dev = slot_global - dev_of_edge * EPAD
    return dict(order=order, sdst=sdst, dev_of_edge=dev_of_edge,
                slot_in_dev=slot_in_dev)


def _device_inputs(P, src, dstf, edge_dst, d):
    mask = P["dev_of_edge"] == d
    slots = P["slot_in_dev"][mask]
    eids = P["order"][mask]

    srcT = np.zeros((EPAD, D), np.float16)
    srcT[slots] = src[eids].astype(np.float16)
    srcT = np.ascontiguousarray(srcT.T)

    dstT = np.zeros((EPAD, D), np.float16)
    dstT[slots] = dstf[edge_dst[eids]].astype(np.float16)
    dstT = np.ascontiguousarray(dstT.T)

    used = np.zeros(EPAD, bool)
    used[slots] = True
    fl = np.ones(EPAD, np.float32)
    sd = P["sdst"][mask]
    seg_start_slots = slots[np.concatenate([[True], sd[1:] != sd[:-1]])]
    fl[seg_start_slots] = 0.0
    prev_used = np.concatenate([[False], used[:-1]])
    run_start = (~used) & (prev_used | (np.arange(EPAD) % F == 0))
    fl[run_start] = 0.0
    fl[np.arange(0, EPAD, F)] = 0.0
    flags = fl.reshape(128, F)

    nxt_reset = np.concatenate([flags[:, 1:], np.zeros((128, 1), np.float32)], axis=1)
    endm = np.where(nxt_reset == 0.0, 1.0, 0.0).astype(np.float32)
    fbwd = np.ascontiguousarray((1.0 - endm)[:, ::-1])

    return dict(srcT=srcT, dstT=dstT, flags=flags, endm=endm, fbwd=fbwd), slots, eids


_CACHE = {}


def run(inputs, trace=False):
    src = np.asarray(inputs["src_feat"], np.float32)
    dstf = np.asarray(inputs["dst_feat"], np.float32)
    edge_dst = np.asarray(inputs["edge_dst"]).astype(np.int64)
    assert src.shape == (N_EDGES, D) and dstf.shape == (N_NODES, D)

    P = _pack(edge_dst)

    wsrc = np.asarray(inputs["W_src"], np.float32).astype(np.float16)
    wdst = np.asarray(inputs["W_dst"], np.float32).astype(np.float16)
    w1 = np.asarray(inputs["W1"], np.float32).astype(np.float16)
    w2v = np.asarray(inputs["W2"], np.float32).astype(np.float16).reshape(D)
    w2pad = np.zeros((D, LGB * 32), np.float16)
    for k in range(LGB):
        w2pad[:, 32 * k + k] = w2v
    bsum = (np.asarray(inputs["b_src"], np.float32)
            + np.asarray(inputs["b_dst"], np.float32)).reshape(D, 1)
    b1 = np.asarray(inputs["b1"], np.float32).reshape(D, 1)
    bexp = np.full((D, 1), float(np.asarray(inputs["b2"]).reshape(-1)[0]) - SHIFT,
                   np.float32)

    in_maps = []
    recov = []
    for d in range(CORES):
        dv, slots, eids = _device_inputs(P, src, dstf, edge_dst, d)
        dv.update(wsrc=wsrc, wdst=wdst, w1=w1, w2pad=w2pad, bsum=bsum, b1=b1,
                  bexp=bexp)
        in_maps.append(dv)
        recov.append((slots, eids))

    if "nc" not in _CACHE:
        _CACHE["nc"] = build_nc()
    nc = _CACHE["nc"]

    try:
        res = run_bass_kernel_spmd(nc, in_maps, list(range(CORES)), trace=trace)
    except ModuleNotFoundError:
        # NTFF profiling hooks unavailable in this environment; run untraced.
        res = run_bass_kernel_spmd(nc, in_maps, list(range(CORES)), trace=False)

    out = np.empty(N_EDGES, np.float32)
    for d in range(CORES):
        slots, eids = recov[d]
        vals = np.asarray(res.results[d]["out"], np.float32).reshape(-1)
        out[eids] = vals[slots]
    _CACHE["exec_time_ns"] = res.exec_time_ns
    _CACHE["trace_path"] = (res.instructions_and_trace or (None, None))[1]
    return out[:, None]


def kernel(**inputs):
    return run(inputs, trace=bool(os.environ.get("BASS_TRACE")))
